# revision 1
# baseline (speedup 1.0000x reference)
"""AttnBlock (GroupNorm -> single-head attention over H*W -> proj -> residual)
for Trainium2, 8 NeuronCores via SPMD.

Sharding: core = b*4 + qi  (b = batch 0/1, qi = query-quarter 0..3).
Each core computes GN stats for its batch, folds the GN affine into the
QKV weights, builds K [C,N] and V^T [N,C] for the full sequence, and runs
attention for its 1024 query rows with scores in [J, I] layout (softmax
along PSUM partitions via a PE ones-matmul; no transposes, no max
subtraction -- scores are O(5) for this model family so exp is safe).
All big matmuls run as float32r (full PE rate, ~1e-4 rel err).
"""
import sys

sys.path.insert(0, '/opt/trn_rl_repo')

import numpy as np

C = 512
NG = 32
EPS = 1e-6
B = 2
N = 4096          # H*W
NQ = 1024         # query quarter
NCT = 4           # C // 128
NJB = 8           # x-stream blocks of 512
NJS = 32          # J subtiles of 128
JB = 512

_cache = {}


def _legalize_waits(nc, mybir):
    """Codegen allows exactly ONE sync wait per instruction. Hoist excess
    waits onto preceding same-engine NoOps (semantics preserving)."""
    gen = 0
    for f in nc.m.functions:
        for bb in f.blocks:
            insts = list(bb.instructions)
            out = []
            changed = False
            for inst in insts:
                si = inst.sync_info
                waits = list(si.on_wait) if si and si.on_wait else []
                if len(waits) > 1:
                    for w in waits[:-1]:
                        gen += 1
                        nop = mybir.InstNoOp(
                            name=f"waitnop_{gen}", ins=[], outs=[],
                            engine=inst.engine)
                        nop.sync_info = mybir.SyncInfo(on_wait=[w], on_update=[])
                        out.append(nop)
                    inst.sync_info = mybir.SyncInfo(
                        on_wait=[waits[-1]],
                        on_update=list(si.on_update) if si and si.on_update else [])
                    changed = True
                out.append(inst)
            if changed:
                bb.instructions = out


def _build():
    import concourse.bass as bass
    import concourse.tile as tile
    from concourse import mybir
    from contextlib import ExitStack

    f32r = mybir.dt.float32r
    f32 = mybir.dt.float32
    AF = mybir.ActivationFunctionType

    nc = bass.Bass(trn_type="TRN2", target_bir_lowering=False, debug=False)

    x = nc.dram_tensor("x", [C, N], f32, kind="ExternalInput").ap()
    xq = nc.dram_tensor("xq", [C, NQ], f32, kind="ExternalInput").ap()
    wqk = nc.dram_tensor("wqk", [C, C], f32, kind="ExternalInput").ap()
    wvT = nc.dram_tensor("wvT", [C, C], f32, kind="ExternalInput").ap()
    woT = nc.dram_tensor("woT", [C, C], f32, kind="ExternalInput").ap()
    gamma4 = nc.dram_tensor("gamma4", [NCT, 128, 1], f32, kind="ExternalInput").ap()
    beta4 = nc.dram_tensor("beta4", [NCT, 128, 1], f32, kind="ExternalInput").ap()
    hq4 = nc.dram_tensor("hq4", [NCT, 128, 1], f32, kind="ExternalInput").ap()
    bv4 = nc.dram_tensor("bv4", [NCT, 128, 1], f32, kind="ExternalInput").ap()
    bo4 = nc.dram_tensor("bo4", [NCT, 128, 1], f32, kind="ExternalInput").ap()
    gmask = nc.dram_tensor("gmask", [128, 8], f32, kind="ExternalInput").ap()
    onesc = nc.dram_tensor("onesc", [128, 128], f32, kind="ExternalInput").ap()
    bmask = nc.dram_tensor("bmask", [8, 128], f32, kind="ExternalInput").ap()
    out = nc.dram_tensor("out", [C, NQ], f32, kind="ExternalOutput").ap()

    dma = nc.sync.dma_start

    with tile.TileContext(nc) as tc, ExitStack() as top:
        consts = top.enter_context(tc.tile_pool(name="consts", bufs=1))
        xpool = top.enter_context(tc.tile_pool(name="xpool", bufs=1))
        wpool = top.enter_context(tc.tile_pool(name="wpool", bufs=1))
        spool = top.enter_context(tc.tile_pool(name="spool", bufs=1))
        oaccp = top.enter_context(tc.tile_pool(name="oaccp", bufs=1))
        qkp = top.enter_context(tc.tile_pool(name="qkp", bufs=1))
        vbp = top.enter_context(tc.tile_pool(name="vbp", bufs=1))
        ptp = top.enter_context(tc.tile_pool(name="ptp", bufs=4))
        osb = top.enter_context(tc.tile_pool(name="osb", bufs=1))
        rp = top.enter_context(tc.tile_pool(name="rp", bufs=1))
        xrp = top.enter_context(tc.tile_pool(name="xrp", bufs=4))
        outp = top.enter_context(tc.tile_pool(name="outp", bufs=2))
        xsp = top.enter_context(tc.tile_pool(name="xsp", bufs=4))
        ps_o = top.enter_context(tc.tile_pool(name="ps_o", bufs=2, space="PSUM"))
        ps_st = top.enter_context(tc.tile_pool(name="ps_st", bufs=2, space="PSUM"))

        # ---- constants ----
        gm = consts.tile([128, 8], f32r, tag="gm", name="gm")
        dma(gm[:], gmask.bitcast(f32r))
        bm = consts.tile([8, 128], f32r, tag="bm", name="bm")
        dma(bm[:], bmask.bitcast(f32r))
        ones2 = consts.tile([128, 2], f32r, tag="ones2", name="ones2")
        dma(ones2[:], onesc[:, 0:2].bitcast(f32r))
        onesr = consts.tile([2, 128], f32r, tag="onesr", name="onesr")
        dma(onesr[:], onesc[0:2, :].bitcast(f32r))
        epst = consts.tile([128, 1], f32, tag="epst", name="epst")
        nc.vector.memset(epst[:], EPS)
        gam = [consts.tile([128, 1], f32, tag=f"gam{i}", name=f"gam{i}") for i in range(NCT)]
        bet = [consts.tile([128, 1], f32, tag=f"bet{i}", name=f"bet{i}") for i in range(NCT)]
        hqc = [consts.tile([128, 1], f32, tag=f"hqc{i}", name=f"hqc{i}") for i in range(NCT)]
        bvc = [consts.tile([128, 1], f32, tag=f"bvc{i}", name=f"bvc{i}") for i in range(NCT)]
        boc = [consts.tile([128, 1], f32, tag=f"boc{i}", name=f"boc{i}") for i in range(NCT)]
        for i in range(NCT):
            dma(gam[i][:], gamma4[i, :, :])
            dma(bet[i][:], beta4[i, :, :])
            dma(hqc[i][:], hq4[i, :, :])
            dma(bvc[i][:], bv4[i, :, :])
            dma(boc[i][:], bo4[i, :, :])
        s_col = [consts.tile([128, 1], f32, tag=f"scol{i}", name=f"scol{i}") for i in range(NCT)]
        t2 = [consts.tile([128, 2], f32r, tag=f"t2{i}", name=f"t2{i}") for i in range(NCT)]
        u_s = [consts.tile([128, 1], f32, tag=f"us{i}", name=f"us{i}") for i in range(NCT)]
        su = [consts.tile([128, 1], f32, tag=f"su{i}", name=f"su{i}") for i in range(NCT)]
        bvt = [consts.tile([128, 2], f32r, tag=f"bvt{i}", name=f"bvt{i}") for i in range(NCT)]
        bo_s = [consts.tile([128, 1], f32, tag=f"bos{i}", name=f"bos{i}") for i in range(NCT)]

        # x resident: 16 chunk tiles [128, 1024] for fine-grained DMA deps
        x_t = [[xpool.tile([128, 1024], f32r, tag=f"x{ci}_{ch}", name=f"x{ci}_{ch}")
                for ch in range(4)] for ci in range(NCT)]
        for ci in range(NCT):
            for ch in range(4):
                eng = dma if (ci * 4 + ch) % 2 == 0 else nc.gpsimd.dma_start
                eng(x_t[ci][ch][:],
                    x[ci * 128:(ci + 1) * 128,
                      ch * 1024:(ch + 1) * 1024].bitcast(f32r))

        wqk_t = [wpool.tile([128, C], f32r, tag=f"wqk{i}", name=f"wqk{i}") for i in range(NCT)]
        wv_t = [wpool.tile([128, C], f32r, tag=f"wv{i}", name=f"wv{i}") for i in range(NCT)]
        wo_t = [wpool.tile([128, C], f32r, tag=f"wo{i}", name=f"wo{i}") for i in range(NCT)]
        dma2 = nc.gpsimd.dma_start
        for i in range(NCT):
            r = slice(i * 128, (i + 1) * 128)
            dma(wqk_t[i][:], wqk[r, :].bitcast(f32r))
            dma(wv_t[i][:], wvT[r, :].bitcast(f32r))
            dma2(wo_t[i][:], woT[r, :].bitcast(f32r))

        # ---- P0: bn_stats over resident x ----
        stats3 = [spool.tile([128, 8, 6], f32, tag=f"st3{i}", name=f"st3{i}")
                  for i in range(NCT)]
        for ci in range(NCT):
            for ch in range(4):
                for h in range(2):
                    nc.vector.bn_stats(
                        out=stats3[ci][:, ch * 2 + h, :],
                        in_=x_t[ci][ch][:, h * 512:(h + 1) * 512].bitcast(f32))

        # ---- P1: stats -> per-channel scale/shift ----
        stats2 = [spool.tile([128, 2], f32r, tag=f"st2{i}", name=f"st2{i}") for i in range(NCT)]
        ssum = ps_st.tile([8, 8], f32, tag="st", name="ssum")
        for ci in range(NCT):
            mv = spool.tile([128, 2], f32, tag="mv", name="mv")
            nc.vector.bn_aggr(out=mv[:], in_=stats3[ci][:, :, :])
            m2 = spool.tile([128, 1], f32, tag="m2", name="m2")
            nc.vector.tensor_mul(m2[:], mv[:, 0:1], mv[:, 0:1])
            nc.vector.tensor_add(stats2[ci][:, 1:2], mv[:, 1:2], m2[:])
            nc.vector.tensor_copy(stats2[ci][:, 0:1], mv[:, 0:1])
            nc.tensor.matmul(ssum[0:8, 2 * ci:2 * ci + 2], gm[:],
                             stats2[ci][:], start=True, stop=True)
        sg = spool.tile([8, 8], f32, tag="sg", name="sg")
        nc.scalar.activation(sg[:], ssum[:], AF.Copy)
        m2g = spool.tile([8, 4], f32, tag="m2g", name="m2g")
        nc.vector.tensor_mul(m2g[:], sg[:, 0:8:2], sg[:, 0:8:2])
        varg = spool.tile([8, 4], f32, tag="varg", name="varg")
        nc.vector.tensor_sub(varg[:], sg[:, 1:8:2], m2g[:])
        sq = spool.tile([8, 4], f32, tag="sq", name="sq")
        nc.scalar.activation(sq[:], varg[:], AF.Sqrt, bias=epst[0:8, :],
                             scale=1.0)
        r0 = spool.tile([8, 4], f32, tag="r0", name="r0")
        nc.vector.reciprocal(r0[:], sq[:])
        aeps = spool.tile([8, 4], f32, tag="aeps", name="aeps")
        nc.vector.tensor_scalar_add(aeps[:], varg[:], EPS)
        rr = spool.tile([8, 4], f32, tag="rr", name="rr")
        nc.vector.tensor_mul(rr[:], r0[:], r0[:])
        nc.vector.tensor_mul(rr[:], rr[:], aeps[:])
        nc.vector.tensor_scalar(out=rr[:], in0=rr[:], scalar1=-0.5,
                                scalar2=1.5, op0=mybir.AluOpType.mult,
                                op1=mybir.AluOpType.add)
        rstdg = spool.tile([8, 4], f32, tag="rstdg", name="rstdg")
        nc.vector.tensor_mul(rstdg[:], r0[:], rr[:])
        mrstd = spool.tile([8, 8], f32r, tag="mrstd", name="mrstd")
        nc.vector.tensor_copy(mrstd[:, 0:8:2], sg[:, 0:8:2])
        nc.vector.tensor_copy(mrstd[:, 1:8:2], rstdg[:])
        for ci in range(NCT):
            pc_ps = ps_st.tile([128, 2], f32, tag="st", name="pc_ps")
            nc.tensor.matmul(pc_ps[:], bm[:], mrstd[:, 2 * ci:2 * ci + 2],
                             start=True, stop=True)
            perch = spool.tile([128, 2], f32, tag=f"pch{ci}", name=f"pch{ci}")
            nc.scalar.activation(perch[:], pc_ps[:], AF.Copy)
            nc.vector.tensor_mul(s_col[ci][:], perch[:, 1:2], gam[ci][:])
            tmp = spool.tile([128, 1], f32, tag="tmp1", name="tmp1")
            nc.vector.tensor_mul(tmp[:], perch[:, 0:1], s_col[ci][:])
            tcol = spool.tile([128, 1], f32, tag=f"tcol{ci}", name=f"tcol{ci}")
            nc.vector.tensor_sub(tcol[:], bet[ci][:], tmp[:])
            nc.vector.tensor_copy(t2[ci][:, 0:1], tcol[:])
            nc.vector.tensor_copy(t2[ci][:, 1:2], tcol[:])

        # ---- P2: bias folds (raw weights), then scale weights ----
        for ot in range(NCT):
            o_sl = slice(ot * 128, (ot + 1) * 128)
            pq = ps_st.tile([128, 2], f32, tag="st", name="pq")
            for ci in range(NCT):
                nc.tensor.matmul(pq[:], wqk_t[ci][:, o_sl], t2[ci][:],
                                 start=(ci == 0), stop=(ci == NCT - 1))
            nc.scalar.activation(u_s[ot][:], pq[:, 0:1], AF.Identity,
                                 bias=hqc[ot][:], scale=1.0)
            nc.vector.tensor_mul(su[ot][:], u_s[ot][:], s_col[ot][:])
        for mt in range(NCT):
            o_sl = slice(mt * 128, (mt + 1) * 128)
            pv = ps_st.tile([128, 2], f32, tag="st", name="pv")
            for ci in range(NCT):
                nc.tensor.matmul(pv[:], wv_t[ci][:, o_sl], t2[ci][:],
                                 start=(ci == 0), stop=(ci == NCT - 1))
            nc.scalar.activation(bvt[mt][:], pv[:], AF.Identity,
                                 bias=bvc[mt][:], scale=1.0)
        for ot in range(NCT):
            o_sl = slice(ot * 128, (ot + 1) * 128)
            pb = ps_st.tile([128, 2], f32, tag="st", name="pb")
            for ci in range(NCT):
                nc.tensor.matmul(pb[:], wo_t[ci][:, o_sl], bvt[ci][:],
                                 start=(ci == 0), stop=(ci == NCT - 1))
            nc.scalar.activation(bo_s[ot][:], pb[:, 0:1], AF.Identity,
                                 bias=boc[ot][:], scale=1.0)
        for ci in range(NCT):
            for wt in (wqk_t, wv_t):
                nc.scalar.activation(wt[ci][:], wt[ci][:].bitcast(f32),
                                     AF.Identity, scale=s_col[ci][:])

        # ---- qk = diag(s)(M^T diag(s) xq + u): K and Q projections fused ----
        qk_t = [qkp.tile([128, NQ], f32r, tag=f"qk{i}", name=f"qk{i}")
                for i in range(NCT)]
        for ib in range(2):
            i0 = ib * 512
            xqb = [None] * NCT
            for cj in range(NCT):
                xqb[cj] = xsp.tile([128, 512], f32r, tag="xqb", name="xqb")
                (dma if cj % 2 == 0 else nc.gpsimd.dma_start)(
                    xqb[cj][:],
                    xq[cj * 128:(cj + 1) * 128, i0:i0 + 512].bitcast(f32r))
            for mt in range(NCT):
                m_sl = slice(mt * 128, (mt + 1) * 128)
                pqk = ps_o.tile([128, 512], f32, tag="op", name="pqk")
                for cj in range(NCT):
                    nc.tensor.matmul(pqk[:], wqk_t[cj][:, m_sl], xqb[cj][:],
                                     start=(cj == 0), stop=(cj == NCT - 1))
                nc.scalar.activation(qk_t[mt][:, i0:i0 + 512], pqk[:],
                                     AF.Identity, scale=s_col[mt][:],
                                     bias=su[mt][:])

        # ---- fused V^T projection + S/O accumulation per J-block ----
        o_acc = [oaccp.tile([128, NQ], f32, tag=f"oacc{i}", name=f"oacc{i}")
                 for i in range(NCT)]
        z_acc = [oaccp.tile([1, 512], f32, tag=f"zacc{ic}", name=f"zacc{ic}")
                 for ic in range(2)]
        for jb in range(NJB):
            j0 = jb * JB
            ch, hf = j0 // 1024, (j0 % 1024) // 512
            xsl = slice(hf * 512, hf * 512 + 512)
            v_b = [vbp.tile([128, C], f32r, tag=f"vb{j}", name=f"vb{j}")
                   for j in range(4)]
            for jt in range(4):
                j_sl = slice(hf * 512 + jt * 128, hf * 512 + jt * 128 + 128)
                pvt = ps_o.tile([128, C], f32, tag="op", name="pvt")
                for ci in range(NCT):
                    nc.tensor.matmul(pvt[:], x_t[ci][ch][:, j_sl], wv_t[ci][:],
                                     start=(ci == 0), stop=(ci == NCT - 1))
                nc.scalar.activation(v_b[jt][:], pvt[:], AF.Copy)
            for ic in range(2):
                i0 = ic * 512
                o_part = [None] * NCT
                z_part = ps_st.tile([2, 512], f32, tag="st", name="z_part")
                prev = None
                for jt in range(4):
                    st_ps = ps_st.tile([128, 512], f32, tag="st", name="st_ps")
                    x_sl = slice(hf * 512 + jt * 128, hf * 512 + jt * 128 + 128)
                    for ci in range(NCT):
                        nc.tensor.matmul(st_ps[:], x_t[ci][ch][:, x_sl],
                                         qk_t[ci][:, i0:i0 + 512],
                                         start=(ci == 0), stop=(ci == NCT - 1))
                    pt = ptp.tile([128, 512], f32r, tag="pt", name="pt")
                    nc.scalar.activation(pt[:], st_ps[:], AF.Exp)
                    if prev is not None:
                        _consume2(nc, prev, z_part, o_part, ones2, v_b)
                    prev = (pt, jt)
                    if jt == 0:
                        for mt in range(NCT):
                            o_part[mt] = ps_o.tile([128, 512], f32,
                                                   tag="opart", name="opart",
                                                   bufs=4)
                _consume2(nc, prev, z_part, o_part, ones2, v_b)
                if jb == 0:
                    nc.vector.tensor_copy(z_acc[ic][:], z_part[0:1, :])
                else:
                    nc.vector.tensor_add(z_acc[ic][:], z_acc[ic][:],
                                         z_part[0:1, :])
                for mt in range(NCT):
                    if jb == 0:
                        nc.vector.tensor_copy(o_acc[mt][:, i0:i0 + 512],
                                              o_part[mt][:])
                    else:
                        nc.vector.tensor_add(o_acc[mt][:, i0:i0 + 512],
                                             o_acc[mt][:, i0:i0 + 512],
                                             o_part[mt][:])

        # ---- normalize + output ----
        for ic in range(2):
            i0 = ic * 512
            r_sb = rp.tile([1, 512], f32r, tag="rsb", name="rsb")
            with nc.allow_low_precision(reason="f32r label on exact DVE divide"):
                nc.vector.reciprocal(r_sb[:], z_acc[ic][:])
            r_ps = ps_st.tile([128, 512], f32, tag="st", name="r_ps")
            nc.tensor.matmul(r_ps[:], onesr[0:1, :], r_sb[:],
                             start=True, stop=True)
            R = rp.tile([128, 512], f32, tag="R", name="R")
            nc.scalar.activation(R[:], r_ps[:], AF.Copy)
            o_sb = [osb.tile([128, 512], f32r, tag=f"osb{mt}", name=f"osb{mt}")
                    for mt in range(NCT)]
            for mt in range(NCT):
                nc.vector.tensor_mul(o_sb[mt][:], o_acc[mt][:, i0:i0 + 512],
                                     R[:])
            for ot in range(NCT):
                o_sl = slice(ot * 128, (ot + 1) * 128)
                f_ps = ps_o.tile([128, 512], f32, tag="op", name="f_ps")
                for ct in range(NCT):
                    nc.tensor.matmul(f_ps[:], wo_t[ct][:, o_sl], o_sb[ct][:],
                                     start=(ct == 0), stop=(ct == NCT - 1))
                ot_sb = outp.tile([128, 512], f32, tag="outsb", name="outsb")
                nc.scalar.activation(ot_sb[:], f_ps[:], AF.Identity,
                                     bias=bo_s[ot][:], scale=1.0)
                xr = xrp.tile([128, 512], f32, tag="xr", name="xr")
                dma(xr[:], xq[ot * 128:(ot + 1) * 128, i0:i0 + 512])
                nc.vector.tensor_add(ot_sb[:], ot_sb[:], xr[:])
                dma(out[ot * 128:(ot + 1) * 128, i0:i0 + 512], ot_sb[:])

    _legalize_waits(nc, mybir)
    return nc


def _consume2(nc, prev, z_ps, o_part, ones2, v_b):
    pt, jt = prev
    nc.tensor.matmul(z_ps[:], ones2[:], pt[:],
                     start=(jt == 0), stop=(jt == 3))
    for mt in range(len(o_part)):
        o_sl = slice(mt * 128, (mt + 1) * 128)
        nc.tensor.matmul(o_part[mt][:], v_b[jt][:, o_sl], pt[:],
                         start=(jt == 0), stop=(jt == 3))


def kernel(**inputs):
    import concourse.bass  # noqa: F401  (ensures repo import works)
    from concourse.bass_utils import run_bass_kernel_spmd

    x = np.asarray(inputs["x"], dtype=np.float32)
    gamma = np.asarray(inputs["gamma"], np.float32)
    beta = np.asarray(inputs["beta"], np.float32)
    wq = np.asarray(inputs["wq"], np.float32)
    bq = np.asarray(inputs["bq"], np.float32)
    wk = np.asarray(inputs["wk"], np.float32)
    wv = np.asarray(inputs["wv"], np.float32)
    wo = np.asarray(inputs["wo"], np.float32)
    bv = np.asarray(inputs["bv"], np.float32)
    bo = np.asarray(inputs["bo"], np.float32)

    Bb, Cc, H, W = x.shape
    scale = Cc ** (-0.5)
    xf = x.reshape(Bb, Cc, H * W)

    wqk_h = np.ascontiguousarray(scale * (wq.T @ wk))
    hq4 = np.ascontiguousarray((scale * (wk.T @ bq)).reshape(NCT, 128, 1))
    wvT = np.ascontiguousarray(wv.T)
    woT = np.ascontiguousarray(wo.T)
    g4 = np.ascontiguousarray(gamma.reshape(NCT, 128, 1))
    b4 = np.ascontiguousarray(beta.reshape(NCT, 128, 1))
    bv4 = np.ascontiguousarray(bv.reshape(NCT, 128, 1))
    bo4 = np.ascontiguousarray(bo.reshape(NCT, 128, 1))
    gmask = np.zeros((128, 8), np.float32)
    for p in range(128):
        gmask[p, p // 16] = 1.0 / 16.0
    bmask = np.zeros((8, 128), np.float32)
    for p in range(128):
        bmask[p // 16, p] = 1.0

    if "nc" not in _cache:
        _cache["nc"] = _build()
    nc = _cache["nc"]

    in_maps = []
    for core in range(8):
        b, qi = core // 4, core % 4
        xb = np.ascontiguousarray(xf[b])
        xqc = np.ascontiguousarray(xf[b][:, qi * NQ:(qi + 1) * NQ])
        in_maps.append({
            "x": xb, "xq": xqc, "wqk": wqk_h, "wvT": wvT,
            "woT": woT, "gamma4": g4, "beta4": b4, "hq4": hq4, "bv4": bv4,
            "bo4": bo4, "gmask": gmask, "bmask": bmask,
            "onesc": _ONES,
        })

    global _last_in_maps
    _last_in_maps = in_maps
    res = run_bass_kernel_spmd(nc, in_maps, core_ids=list(range(8)))
    outf = np.empty((Bb, Cc, H * W), np.float32)
    for core in range(8):
        b, qi = core // 4, core % 4
        outf[b][:, qi * NQ:(qi + 1) * NQ] = res.results[core]["out"]
    return outf.reshape(Bb, Cc, H, W)


_ONES = np.ones((128, 128), np.float32)



# revision 5
# speedup vs baseline: 1.7601x; 1.7601x over previous
"""AttnBlock (GroupNorm -> single-head attention over H*W -> proj -> residual)
for Trainium2, 8 NeuronCores via SPMD.

Sharding: core = b*4 + qi (b = batch 0/1, qi = query-quarter 0..3).  The host
permutes each core's x columns so its query quarter is always column-chunk 0
(attention + GN stats are permutation-invariant over the key axis), so one
compiled kernel serves all 8 cores.

v2: every large matmul runs as fp8e4 (e4m3) with MatmulPerfMode.DoubleRow
(K=256 per instruction, 0.5 cycles/row).  Each fp8 tensor carries a
power-of-2 pre-scale placing it in e4m3's normal range (wq^T wk entries are
~2^-9 -- subnormal unscaled, which destroys relative precision); inverse
scales fold into existing activation scale operands.  wqk = wq^T wk ships as
a two-term fp8 sum (value + quantization residual, effectively ~fp16): it is
the dominant quantization error source.  Scores stay in [J, I] layout; the
softmax denominator comes from an all-ones fp8 DoubleRow matmul whose lhsT
free dim is 256, broadcasting z to all 128 partitions so normalization needs
no separate PE broadcast.  The j-side GN affine cancels in softmax; the
i-side affine folds into the qk bias.  O accumulates over the full sequence
in PSUM; scores/exp phases and O/Z phases alternate so the 8 PSUM banks
suffice (3 score rotation + 4 V/O + 1 scratch).
"""
import sys

sys.path.insert(0, '/opt/trn_rl_repo')

import numpy as np
import ml_dtypes

C = 512
NG = 32
EPS = 1e-6
B = 2
N = 4096          # H*W
NQ = 1024         # query quarter
NCT = 4           # C // 128
NCP = 2           # C // 256 (DoubleRow c-pairs)
NJP = 16          # j-pairs of 256

SHIFT = 2.5
A_WQK = 1024.0
A_XJ = 2.0
A_XQS = 16.0
A_QK = 128.0
A_WV = 8.0        # A_XJ * A_WV must equal A_V so the V cast is a pure copy
A_V = 16.0
A_O = 16.0        # must equal A_V (keeps the 1/z fold scale-free)
A_T = 256.0
A_BVT = 256.0
A_WO = 64.0

_cache = {}


def _legalize_waits(nc, mybir):
    """Codegen allows exactly ONE sync wait per instruction. Hoist excess
    waits onto preceding same-engine NoOps (semantics preserving)."""
    gen = 0
    for f in nc.m.functions:
        for bb in f.blocks:
            insts = list(bb.instructions)
            out = []
            changed = False
            for inst in insts:
                si = inst.sync_info
                waits = list(si.on_wait) if si and si.on_wait else []
                if len(waits) > 1:
                    for w in waits[:-1]:
                        gen += 1
                        nop = mybir.InstNoOp(
                            name=f"waitnop_{gen}", ins=[], outs=[],
                            engine=inst.engine)
                        nop.sync_info = mybir.SyncInfo(on_wait=[w], on_update=[])
                        out.append(nop)
                    inst.sync_info = mybir.SyncInfo(
                        on_wait=[waits[-1]],
                        on_update=list(si.on_update) if si and si.on_update else [])
                    changed = True
                out.append(inst)
            if changed:
                bb.instructions = out


def _build():
    import concourse.bass as bass
    import concourse.tile as tile
    from concourse import mybir
    from contextlib import ExitStack

    f32r = mybir.dt.float32r
    f32 = mybir.dt.float32
    f8 = mybir.dt.float8e4
    bf = mybir.dt.bfloat16
    u8 = mybir.dt.uint8
    u16 = mybir.dt.uint16
    AF = mybir.ActivationFunctionType
    DR = mybir.MatmulPerfMode.DoubleRow
    MUL = mybir.AluOpType.mult
    ADD = mybir.AluOpType.add

    nc = bass.Bass(trn_type="TRN2", target_bir_lowering=False, debug=False)

    x = nc.dram_tensor("x", [C, N], f32, kind="ExternalInput").ap()
    wqk8a = nc.dram_tensor("wqk8a", [NCP, 128, 2, 512], u8, kind="ExternalInput").ap()
    wqk8b = nc.dram_tensor("wqk8b", [NCP, 128, 2, 512], u8, kind="ExternalInput").ap()
    wo8d = nc.dram_tensor("wo8d", [NCP, 128, 2, 512], u8, kind="ExternalInput").ap()
    wvbf = nc.dram_tensor("wvbf", [C, C], u16, kind="ExternalInput").ap()
    gamma4 = nc.dram_tensor("gamma4", [NCT, 128, 1], f32, kind="ExternalInput").ap()
    beta4 = nc.dram_tensor("beta4", [NCT, 128, 1], f32, kind="ExternalInput").ap()
    hq4 = nc.dram_tensor("hq4", [NCT, 128, 1], f32, kind="ExternalInput").ap()
    bv4 = nc.dram_tensor("bv4", [NCT, 128, 1], f32, kind="ExternalInput").ap()
    bo4 = nc.dram_tensor("bo4", [NCT, 128, 1], f32, kind="ExternalInput").ap()
    gmask = nc.dram_tensor("gmask", [128, 8], f32, kind="ExternalInput").ap()
    bmask = nc.dram_tensor("bmask", [8, 128], f32, kind="ExternalInput").ap()
    ones8w = nc.dram_tensor("ones8w", [128, 2, 128], u8, kind="ExternalInput").ap()
    out = nc.dram_tensor("out", [C, NQ], f32, kind="ExternalOutput").ap()

    dma = nc.sync.dma_start
    dma2 = nc.gpsimd.dma_start
    qi_ch = 0  # host permutes x so the query quarter is chunk 0

    with tile.TileContext(nc) as tc, ExitStack() as top:
        consts = top.enter_context(tc.tile_pool(name="consts", bufs=1))
        xpool = top.enter_context(tc.tile_pool(name="xpool", bufs=1))
        x8pool = top.enter_context(tc.tile_pool(name="x8pool", bufs=1))
        wpool = top.enter_context(tc.tile_pool(name="wpool", bufs=1))
        w8pool = top.enter_context(tc.tile_pool(name="w8pool", bufs=1))
        qk8p = top.enter_context(tc.tile_pool(name="qk8p", bufs=1))
        xqsp = top.enter_context(tc.tile_pool(name="xqsp", bufs=1))
        v8p = top.enter_context(tc.tile_pool(name="v8p", bufs=1))
        ptp = top.enter_context(tc.tile_pool(name="ptp", bufs=1))
        spool = top.enter_context(tc.tile_pool(name="spool", bufs=1))
        rp = top.enter_context(tc.tile_pool(name="rp", bufs=2))
        osbp = top.enter_context(tc.tile_pool(name="osbp", bufs=2))
        outp = top.enter_context(tc.tile_pool(name="outp", bufs=4))
        # PSUM: 1 (scratch chain) + 4 (V pairs / O pairs / fps) + 3 (scores)
        ps_q = top.enter_context(tc.tile_pool(name="ps_q", bufs=1, space="PSUM"))
        ps_v = top.enter_context(tc.tile_pool(name="ps_v", bufs=2, space="PSUM"))
        ps_sw = top.enter_context(tc.tile_pool(name="ps_sw", bufs=1, space="PSUM"))

        def qtile(shape, name):
            return ps_q.tile(shape, f32, tag="q", name=name)

        def vtile(name):
            return ps_v.tile([128, 2, 512], f32, tag="pvt", name=name)

        def swtile(name):
            return ps_sw.tile([128, 3, 512], f32, tag="st3", name=name)

        # ---- small constants (gpsimd DMA queue) ----
        gm = consts.tile([128, 8], f32r, tag="gm", name="gm")
        dma2(gm[:], gmask.bitcast(f32r))
        bm = consts.tile([8, 128], f32r, tag="bm", name="bm")
        dma2(bm[:], bmask.bitcast(f32r))
        ones8 = consts.tile([128, 2, 128], f8, tag="ones8", name="ones8")
        dma2(ones8[:], ones8w.bitcast(f8))
        epst = consts.tile([128, 1], f32, tag="epst", name="epst")
        nc.vector.memset(epst[:], EPS)
        shiftb = consts.tile([128, 1], f32, tag="shiftb", name="shiftb")
        nc.vector.memset(shiftb[:], -SHIFT)
        gam = [consts.tile([128, 1], f32, tag=f"gam{i}", name=f"gam{i}") for i in range(NCT)]
        bet = [consts.tile([128, 1], f32, tag=f"bet{i}", name=f"bet{i}") for i in range(NCT)]
        hqc = [consts.tile([128, 1], f32, tag=f"hqc{i}", name=f"hqc{i}") for i in range(NCT)]
        bvc = [consts.tile([128, 1], f32, tag=f"bvc{i}", name=f"bvc{i}") for i in range(NCT)]
        boc = [consts.tile([128, 1], f32, tag=f"boc{i}", name=f"boc{i}") for i in range(NCT)]
        for i in range(NCT):
            dma2(gam[i][:], gamma4[i, :, :])
            dma2(bet[i][:], beta4[i, :, :])
            dma2(hqc[i][:], hq4[i, :, :])
            dma2(bvc[i][:], bv4[i, :, :])
            dma2(boc[i][:], bo4[i, :, :])
        wqa_t = [w8pool.tile([128, 2, 512], f8, tag=f"wqa{p}", name=f"wqa{p}")
                 for p in range(NCP)]
        wqb_t = [w8pool.tile([128, 2, 512], f8, tag=f"wqb{p}", name=f"wqb{p}")
                 for p in range(NCP)]
        wo_t = [w8pool.tile([128, 2, 512], f8, tag=f"wo8{p}", name=f"wo8{p}")
                for p in range(NCP)]
        for p in range(NCP):
            dma2(wqa_t[p][:], wqk8a[p, :, :, :].bitcast(f8))
            dma2(wqb_t[p][:], wqk8b[p, :, :, :].bitcast(f8))
            dma2(wo_t[p][:], wo8d[p, :, :, :].bitcast(f8))

        # ---- x resident (sync DMA) + fp8 cast (ACT) + stats (DVE) ----
        x_t = [[xpool.tile([128, 1024], f32, tag=f"x{ci}_{ch}", name=f"x{ci}_{ch}")
                for ch in range(4)] for ci in range(NCT)]
        x8t = [[x8pool.tile([128, 2, 1024], f8, tag=f"x8{p}_{ch}", name=f"x8{p}_{ch}")
                for ch in range(4)] for p in range(NCP)]
        stats3 = [spool.tile([128, 8, 6], f32, tag=f"st3s{i}", name=f"st3s{i}")
                  for i in range(NCT)]
        for ci in range(NCT):
            for ch in range(4):
                dma(x_t[ci][ch][:], x[ci * 128:(ci + 1) * 128,
                                      ch * 1024:(ch + 1) * 1024])
                nc.scalar.activation(x8t[ci // 2][ch][:, ci % 2, :],
                                     x_t[ci][ch][:], AF.Copy, scale=A_XJ)
                for h in range(2):
                    nc.vector.bn_stats(
                        out=stats3[ci][:, ch * 2 + h, :],
                        in_=x_t[ci][ch][:, h * 512:(h + 1) * 512])
        wv_t = [wpool.tile([128, 512], bf, tag=f"wv{i}", name=f"wv{i}")
                for i in range(NCT)]
        for i in range(NCT):
            dma(wv_t[i][:], wvbf[i * 128:(i + 1) * 128, :].bitcast(bf))

        # ---- P1: stats -> per-channel scale/shift ----
        stats2 = [spool.tile([128, 2], f32r, tag=f"st2{i}", name=f"st2{i}") for i in range(NCT)]
        ssum = qtile([8, 8], "ssum")
        for ci in range(NCT):
            mv = spool.tile([128, 2], f32, tag="mv", name="mv")
            nc.vector.bn_aggr(out=mv[:], in_=stats3[ci][:, :, :])
            m2 = spool.tile([128, 1], f32, tag="m2", name="m2")
            nc.vector.tensor_mul(m2[:], mv[:, 0:1], mv[:, 0:1])
            nc.vector.tensor_add(stats2[ci][:, 1:2], mv[:, 1:2], m2[:])
            nc.vector.tensor_copy(stats2[ci][:, 0:1], mv[:, 0:1])
            nc.tensor.matmul(ssum[0:8, 2 * ci:2 * ci + 2], gm[:],
                             stats2[ci][:], start=(ci == 0), stop=(ci == 3),
                             skip_group_check=True)
        sg = spool.tile([8, 8], f32, tag="sg", name="sg")
        nc.scalar.activation(sg[:], ssum[:], AF.Copy)
        m2g = spool.tile([8, 4], f32, tag="m2g", name="m2g")
        nc.vector.tensor_mul(m2g[:], sg[:, 0:8:2], sg[:, 0:8:2])
        varg = spool.tile([8, 4], f32, tag="varg", name="varg")
        nc.vector.tensor_sub(varg[:], sg[:, 1:8:2], m2g[:])
        sq = spool.tile([8, 4], f32, tag="sq", name="sq")
        nc.scalar.activation(sq[:], varg[:], AF.Sqrt, bias=epst[0:8, :],
                             scale=1.0)
        r0 = spool.tile([8, 4], f32, tag="r0", name="r0")
        nc.vector.reciprocal(r0[:], sq[:])
        aeps = spool.tile([8, 4], f32, tag="aeps", name="aeps")
        nc.vector.tensor_scalar_add(aeps[:], varg[:], EPS)
        rr = spool.tile([8, 4], f32, tag="rr", name="rr")
        nc.vector.tensor_mul(rr[:], r0[:], r0[:])
        nc.vector.tensor_mul(rr[:], rr[:], aeps[:])
        nc.vector.tensor_scalar(out=rr[:], in0=rr[:], scalar1=-0.5,
                                scalar2=1.5, op0=MUL, op1=ADD)
        rstdg = spool.tile([8, 4], f32, tag="rstdg", name="rstdg")
        nc.vector.tensor_mul(rstdg[:], r0[:], rr[:])
        mrstd = spool.tile([8, 8], f32r, tag="mrstd", name="mrstd")
        nc.vector.tensor_copy(mrstd[:, 0:8:2], sg[:, 0:8:2])
        nc.vector.tensor_copy(mrstd[:, 1:8:2], rstdg[:])
        s_col = [spool.tile([128, 1], f32, tag=f"scol{i}", name=f"scol{i}") for i in range(NCT)]
        s16 = [spool.tile([128, 1], f32, tag=f"s16_{i}", name=f"s16_{i}") for i in range(NCT)]
        s8v = [spool.tile([128, 1], f32, tag=f"s8v{i}", name=f"s8v{i}") for i in range(NCT)]
        sdq = [spool.tile([128, 1], f32, tag=f"sdq{i}", name=f"sdq{i}") for i in range(NCT)]
        t2b = [spool.tile([128, 2], bf, tag=f"t2b{i}", name=f"t2b{i}") for i in range(NCT)]
        t28 = [spool.tile([128, 2, 2], f8, tag=f"t28{p}", name=f"t28{p}") for p in range(NCP)]
        for ci in range(NCT):
            pc_ps = qtile([128, 2], f"pc{ci}")
            nc.tensor.matmul(pc_ps[:], bm[:], mrstd[:, 2 * ci:2 * ci + 2],
                             start=True, stop=True)
            perch = spool.tile([128, 2], f32, tag=f"pch{ci}", name=f"pch{ci}")
            nc.scalar.activation(perch[:], pc_ps[:], AF.Copy)
            nc.vector.tensor_mul(s_col[ci][:], perch[:, 1:2], gam[ci][:])
            nc.vector.tensor_scalar_mul(s16[ci][:], s_col[ci][:], A_XQS)
            nc.vector.tensor_scalar_mul(s8v[ci][:], s_col[ci][:], A_WV)
            nc.vector.tensor_scalar_mul(sdq[ci][:], s_col[ci][:],
                                        A_QK / (A_WQK * A_XQS))
            tmp = spool.tile([128, 1], f32, tag="tmp1", name="tmp1")
            nc.vector.tensor_mul(tmp[:], perch[:, 0:1], s_col[ci][:])
            tcol = spool.tile([128, 1], f32, tag=f"tcol{ci}", name=f"tcol{ci}")
            nc.vector.tensor_sub(tcol[:], bet[ci][:], tmp[:])
            nc.vector.tensor_copy(t2b[ci][:, 0:1], tcol[:])
            nc.vector.tensor_copy(t2b[ci][:, 1:2], tcol[:])
            nc.scalar.activation(t28[ci // 2][:, ci % 2, :], t2b[ci][:],
                                 AF.Copy, scale=A_T)

        # ---- u fold: suq = s * A_QK * (wqk^T t + hq) ----
        suq = [spool.tile([128, 1], f32, tag=f"suq{i}", name=f"suq{i}") for i in range(NCT)]
        for ot in range(NCT):
            o_sl = slice(ot * 128, (ot + 1) * 128)
            pq = qtile([128, 2], f"pq{ot}")
            k = 0
            for p in range(NCP):
                for wt in (wqa_t, wqb_t):
                    nc.tensor.matmul(pq[:], wt[p][:, :, o_sl], t28[p][:],
                                     start=(k == 0), stop=(k == 3),
                                     perf_mode=DR)
                    k += 1
            us = spool.tile([128, 1], f32, tag="us", name="us")
            nc.scalar.activation(us[:], pq[:, 0:1], AF.Identity,
                                 bias=hqc[ot][:], scale=A_QK / (A_WQK * A_T))
            nc.vector.tensor_mul(suq[ot][:], us[:], s_col[ot][:])

        # ---- input casts: xqs (query side, scaled), wv8 ----
        xqs = [xqsp.tile([128, 2, 1024], f8, tag=f"xqs{p}", name=f"xqs{p}")
               for p in range(NCP)]
        for ci in range(NCT):
            nc.scalar.activation(xqs[ci // 2][:, ci % 2, 0:512],
                                 x_t[ci][qi_ch][:, 0:512],
                                 AF.Identity, scale=s16[ci][:])
        wv8 = [w8pool.tile([128, 2, 512], f8, tag=f"wv8{p}", name=f"wv8{p}")
               for p in range(NCP)]
        for ci in range(NCT):
            nc.scalar.activation(wv8[ci // 2][:, ci % 2, :], wv_t[ci][:],
                                 AF.Identity, scale=s8v[ci][:])
        for ci in range(NCT):
            nc.scalar.activation(xqs[ci // 2][:, ci % 2, 512:1024],
                                 x_t[ci][qi_ch][:, 512:1024],
                                 AF.Identity, scale=s16[ci][:])

        # ---- qk projection ----
        qk8 = [qk8p.tile([128, 2, 1024], f8, tag=f"qk8{p}", name=f"qk8{p}")
               for p in range(NCP)]

        def qk_mms(ih):
            isl = slice(ih * 512, (ih + 1) * 512)
            for mt in range(NCT):
                m_sl = slice(mt * 128, (mt + 1) * 128)
                pqk = qtile([128, 512], f"pqk{ih}_{mt}")
                k = 0
                for p in range(NCP):
                    for wt in (wqa_t, wqb_t):
                        nc.tensor.matmul(pqk[:], wt[p][:, :, m_sl],
                                         xqs[p][:, :, isl],
                                         start=(k == 0), stop=(k == 3),
                                         perf_mode=DR)
                        k += 1
                nc.vector.tensor_scalar(
                    out=qk8[mt // 2][:, mt % 2, isl], in0=pqk[:],
                    scalar1=sdq[mt][:], scalar2=suq[mt][:],
                    op0=MUL, op1=ADD)

        # ---- V phase + scores/exp(ic0), interleaved emission ----
        v8 = [v8p.tile([128, 2, 512], f8, tag=f"v8_{jp}", name=f"v8_{jp}")
              for jp in range(NJP)]
        pt8 = [[ptp.tile([128, 2, 512], f8, tag=f"pt{ic}_{jp}",
                         name=f"pt{ic}_{jp}") for jp in range(NJP)]
               for ic in range(2)]
        st3 = swtile("st3")

        def v_jp(jp):
            j0 = jp * 256
            ch, jj0 = j0 // 1024, j0 % 1024
            pvt = vtile(f"pvt{jp}")
            for jt in range(2):
                jsl = slice(jj0 + jt * 128, jj0 + jt * 128 + 128)
                for p in range(NCP):
                    nc.tensor.matmul(pvt[:, jt, :], x8t[p][ch][:, :, jsl],
                                     wv8[p][:], start=(p == 0),
                                     stop=(p == NCP - 1), perf_mode=DR)
            nc.vector.tensor_copy(v8[jp][:], pvt[:])

        def se_jp(ic, jp):
            i0 = ic * 512
            isl = slice(i0, i0 + 512)
            j0 = jp * 256
            ch, jj0 = j0 // 1024, j0 % 1024
            bA, bB = (2 * jp) % 3, (2 * jp + 1) % 3
            for jt, bk in ((0, bA), (1, bB)):
                jsl = slice(jj0 + jt * 128, jj0 + jt * 128 + 128)
                for p in range(NCP):
                    nc.tensor.matmul(st3[:, bk, :], x8t[p][ch][:, :, jsl],
                                     qk8[p][:, :, isl],
                                     start=(p == 0), stop=(p == NCP - 1),
                                     perf_mode=DR)
            if bB == bA + 1:
                nc.scalar.activation(pt8[ic][jp][:], st3[:, bA:bA + 2, :],
                                     AF.Exp, bias=shiftb[:],
                                     scale=1.0 / (A_XJ * A_QK))
            else:
                for jt, bk in ((0, bA), (1, bB)):
                    nc.scalar.activation(pt8[ic][jp][:, jt, :], st3[:, bk, :],
                                         AF.Exp, bias=shiftb[:],
                                         scale=1.0 / (A_XJ * A_QK))

        qk_mms(0)
        for jp in range(NJP):
            se_jp(0, jp)
            v_jp(jp)
            if jp == 7:
                qk_mms(1)

        # ---- bvt / bo folds (ACT slot between the two exp phases) ----
        bvt_s = [spool.tile([128, 2], f32, tag=f"bvs{i}", name=f"bvs{i}") for i in range(NCT)]
        bvt8 = [spool.tile([128, 2, 2], f8, tag=f"bv8{p}", name=f"bv8{p}") for p in range(NCP)]
        bo_s = [spool.tile([128, 1], f32, tag=f"bos{i}", name=f"bos{i}") for i in range(NCT)]
        for mt in range(NCT):
            m_sl = slice(mt * 128, (mt + 1) * 128)
            pv = qtile([128, 2], f"pv{mt}")
            for ci in range(NCT):
                nc.tensor.matmul(pv[:], wv_t[ci][:, m_sl], t2b[ci][:],
                                 start=(ci == 0), stop=(ci == NCT - 1))
            nc.scalar.activation(bvt_s[mt][:], pv[:], AF.Identity,
                                 bias=bvc[mt][:], scale=1.0)
            nc.scalar.activation(bvt8[mt // 2][:, mt % 2, :], bvt_s[mt][:],
                                 AF.Copy, scale=A_BVT)
        for ot in range(NCT):
            o_sl = slice(ot * 128, (ot + 1) * 128)
            pb = qtile([128, 2], f"pb{ot}")
            for p in range(NCP):
                nc.tensor.matmul(pb[:], wo_t[p][:, :, o_sl], bvt8[p][:],
                                 start=(p == 0), stop=(p == NCP - 1),
                                 perf_mode=DR)
            nc.scalar.activation(bo_s[ot][:], pb[:, 0:1], AF.Identity,
                                 bias=boc[ot][:], scale=1.0 / (A_WO * A_BVT))

        # ---- z(ic0) + 1/z ----
        z0 = qtile([128, 512], "z0")
        for jp in range(NJP):
            nc.tensor.matmul(z0[:], ones8[:], pt8[0][jp][:],
                             start=(jp == 0), stop=(jp == NJP - 1),
                             perf_mode=DR)
        rinv0 = rp.tile([128, 512], f32, tag="rinv", name="rinv0")
        with nc.allow_low_precision(reason="softmax denominator reciprocal"):
            nc.vector.reciprocal(rinv0[:], z0[:])

        # ---- phase C: O(ic0) accumulation + scores/exp(ic1) + z(ic1) ----
        oAB = [vtile("oA"), vtile("oB")]
        o_ps0 = [oAB[mt // 2][:, mt % 2, :] for mt in range(NCT)]
        z1 = qtile([128, 512], "z1")
        for jp in range(NJP):
            se_jp(1, jp)
            for mt in range(NCT):
                nc.tensor.matmul(o_ps0[mt], v8[jp][:, :, mt * 128:(mt + 1) * 128],
                                 pt8[0][jp][:], start=(jp == 0),
                                 stop=(jp == NJP - 1), perf_mode=DR)
            nc.tensor.matmul(z1[:], ones8[:], pt8[1][jp][:],
                             start=(jp == 0), stop=(jp == NJP - 1),
                             perf_mode=DR)
        rinv1 = rp.tile([128, 512], f32, tag="rinv", name="rinv1")
        with nc.allow_low_precision(reason="softmax denominator reciprocal"):
            nc.vector.reciprocal(rinv1[:], z1[:])

        # ---- phase D: O(ic1) into freed score/z banks + epilogue(ic0) ----
        oD = swtile("oD")
        zD = qtile([128, 512], "zD")
        o_ps1 = [oD[:, 0, :], oD[:, 1, :], oD[:, 2, :], zD[:]]

        def o1_jp(jp):
            for mt in range(NCT):
                nc.tensor.matmul(o_ps1[mt], v8[jp][:, :, mt * 128:(mt + 1) * 128],
                                 pt8[1][jp][:], start=(jp == 0),
                                 stop=(jp == NJP - 1), perf_mode=DR)

        def epilogue(ic, o_ps, rinv):
            i0 = ic * 512
            osb = [osbp.tile([128, 2, 512], f8, tag=f"osb{p}", name=f"osb{ic}{p}")
                   for p in range(NCP)]
            for mt in range(NCT):
                nc.vector.tensor_mul(osb[mt // 2][:, mt % 2, :],
                                     o_ps[mt], rinv[:])
            fps_pair = [None]

            def next_fps(k):
                if k % 2 == 0:
                    fps_pair[0] = vtile(f"fps{ic}_{k // 2}")
                return fps_pair[0][:, k % 2, :]

            for ot in range(NCT):
                o_sl = slice(ot * 128, (ot + 1) * 128)
                f_ps = next_fps(ot)
                for p in range(NCP):
                    nc.tensor.matmul(f_ps, wo_t[p][:, :, o_sl], osb[p][:],
                                     start=(p == 0), stop=(p == NCP - 1),
                                     perf_mode=DR)
                ot_sb = outp.tile([128, 512], f32, tag="outsb", name="outsb")
                nc.scalar.activation(ot_sb[:], f_ps, AF.Identity,
                                     bias=bo_s[ot][:], scale=1.0 / (A_WO * A_O))
                nc.vector.tensor_add(ot_sb[:], ot_sb[:],
                                     x_t[ot][qi_ch][:, i0:i0 + 512])
                dma(out[ot * 128:(ot + 1) * 128, i0:i0 + 512], ot_sb[:])

        for jp in range(11):
            o1_jp(jp)
        epilogue(0, o_ps0, rinv0)
        for jp in range(11, NJP):
            o1_jp(jp)
        epilogue(1, o_ps1, rinv1)

    _legalize_waits(nc, mybir)
    return nc


def _pack_dr(w, alpha):
    """[C, 512] f32 -> [NCP, 128, 2, 512] e4m3 uint8 in DoubleRow layout."""
    E4 = ml_dtypes.float8_e4m3
    q = np.clip(alpha * w, -240, 240).astype(E4)
    return np.ascontiguousarray(
        q.reshape(NCP, 2, 128, 512).transpose(0, 2, 1, 3)).view(np.uint8)


def kernel(**inputs):
    import concourse.bass  # noqa: F401
    from concourse.bass_utils import run_bass_kernel_spmd

    E4 = ml_dtypes.float8_e4m3
    BF = ml_dtypes.bfloat16

    x = np.asarray(inputs["x"], dtype=np.float32)
    gamma = np.asarray(inputs["gamma"], np.float32)
    beta = np.asarray(inputs["beta"], np.float32)
    wq = np.asarray(inputs["wq"], np.float32)
    bq = np.asarray(inputs["bq"], np.float32)
    wk = np.asarray(inputs["wk"], np.float32)
    wv = np.asarray(inputs["wv"], np.float32)
    wo = np.asarray(inputs["wo"], np.float32)
    bv = np.asarray(inputs["bv"], np.float32)
    bo = np.asarray(inputs["bo"], np.float32)

    Bb, Cc, H, W = x.shape
    scale = Cc ** (-0.5)
    xf = x.reshape(Bb, Cc, H * W)

    wqk_h = scale * (wq.T @ wk)
    a8f = np.clip(A_WQK * wqk_h, -240, 240).astype(E4).astype(np.float32)
    wqk8a = _pack_dr(wqk_h, A_WQK)
    wqk8b = np.ascontiguousarray(
        np.clip(A_WQK * wqk_h - a8f, -240, 240).astype(E4)
        .reshape(NCP, 2, 128, 512).transpose(0, 2, 1, 3)).view(np.uint8)
    wo8d = _pack_dr(wo.T, A_WO)
    wvbf = np.ascontiguousarray(wv.T.astype(BF)).view(np.uint16)
    hq4 = np.ascontiguousarray(
        (A_QK * scale * (wk.T @ bq)).reshape(NCT, 128, 1))
    g4 = np.ascontiguousarray(gamma.reshape(NCT, 128, 1))
    b4 = np.ascontiguousarray(beta.reshape(NCT, 128, 1))
    bv4 = np.ascontiguousarray(bv.reshape(NCT, 128, 1))
    bo4 = np.ascontiguousarray(bo.reshape(NCT, 128, 1))
    gmask = np.zeros((128, 8), np.float32)
    for p in range(128):
        gmask[p, p // 16] = 1.0 / 16.0
    bmask = np.zeros((8, 128), np.float32)
    for p in range(128):
        bmask[p // 16, p] = 1.0
    ones8w = np.ones((128, 2, 128), np.float32).astype(E4).view(np.uint8)

    common = {
        "wqk8a": wqk8a, "wqk8b": wqk8b, "wo8d": wo8d, "wvbf": wvbf,
        "gamma4": g4, "beta4": b4, "hq4": hq4, "bv4": bv4, "bo4": bo4,
        "gmask": gmask, "bmask": bmask, "ones8w": ones8w,
    }

    if "nc" not in _cache:
        _cache["nc"] = _build()
    nc = _cache["nc"]

    in_maps = []
    for core in range(8):
        b, qi = core // 4, core % 4
        # permute columns so this core's query quarter is chunk 0; attention
        # and GN stats are permutation-invariant over the key axis, and the
        # output columns are exactly chunk 0's queries.
        xb = xf[b]
        q0 = qi * NQ
        xperm = np.ascontiguousarray(np.concatenate(
            [xb[:, q0:q0 + NQ], xb[:, :q0], xb[:, q0 + NQ:]], axis=1))
        in_maps.append({"x": xperm, **common})

    res = run_bass_kernel_spmd(nc, in_maps, core_ids=list(range(8)))
    outf = np.empty((Bb, Cc, H * W), np.float32)
    for core in range(8):
        b, qi = core // 4, core % 4
        outf[b][:, qi * NQ:(qi + 1) * NQ] = res.results[core]["out"]
    return outf.reshape(Bb, Cc, H, W)


# revision 41
# speedup vs baseline: 2.3083x; 1.3115x over previous
"""AttnBlock (GroupNorm -> single-head attention over H*W -> proj -> residual)
for Trainium2, 8 NeuronCores via SPMD.

Sharding: core = b*4 + qi (b = batch 0/1, qi = query-quarter 0..3).  The host
permutes each core's x columns so its query quarter is always column-chunk 0
(attention + GN stats are permutation-invariant over the key axis), so one
compiled kernel serves all 8 cores.

v3: every large matmul runs as fp8e4 (e4m3) with MatmulPerfMode.DoubleRow
(K=256 per instruction, 0.5 cycles/row).  Each fp8 tensor carries a
power-of-2 pre-scale placing it in e4m3's normal range (wq^T wk entries are
~2^-9 -- subnormal unscaled); inverse scales fold into activation/
tensor_scalar operands.  wqk = wq^T wk ships as a two-term fp8 sum
(value + quantization residual): it is the dominant quantization error
source.  Scores stay in [J, I] layout; the softmax denominator comes from an
all-ones fp8 DoubleRow matmul broadcasting z to all 128 partitions.  The
j-side GN affine cancels in softmax; the i-side affine folds into the qk
bias.  O accumulates over the full 4096-key sequence in PSUM.

Scheduling: one packed DMA per weight class (tiny SWDGE DMAs would occupy
the Pool engine for ~30us); score tiles rotate through a 3-buffer PSUM pool
(per-tile dep tracking pipelines PE scores against ACT exp); qk8/xqs split
per i-half so phase-B readers don't wait on phase-C writers; elementwise
work spread over ACT (exp, i0-half casts), DVE (stats, V copies, i1-half
casts, normalize), Pool (x->fp8 copies, residual adds).
"""
import sys

sys.path.insert(0, '/opt/trn_rl_repo')

import numpy as np
import ml_dtypes

C = 512
NG = 32
EPS = 1e-6
B = 2
N = 4096          # H*W
NQ = 1024         # query quarter
NCT = 4           # C // 128
NCP = 2           # C // 256 (DoubleRow c-pairs)
NJP = 16          # j-pairs of 256

SHIFT = 2.5
A_WQK = 1024.0
A_XJ = 1.0        # A_XJ * A_WV == A_V so the V cast is a pure copy
A_XQS = 16.0
A_QK = 128.0
A_WV = 16.0
A_V = 16.0
A_O = 16.0        # == A_V keeps the 1/z fold scale-free
A_T = 256.0
A_BVT = 256.0
A_WO = 64.0

_cache = {}


def _legalize_waits(nc, mybir):
    """Codegen allows exactly ONE sync wait per instruction. Hoist excess
    waits onto preceding same-engine NoOps (semantics preserving)."""
    gen = 0
    for f in nc.m.functions:
        for bb in f.blocks:
            insts = list(bb.instructions)
            out = []
            changed = False
            for inst in insts:
                si = inst.sync_info
                waits = list(si.on_wait) if si and si.on_wait else []
                if len(waits) > 1:
                    for w in waits[:-1]:
                        gen += 1
                        nop = mybir.InstNoOp(
                            name=f"waitnop_{gen}", ins=[], outs=[],
                            engine=inst.engine)
                        nop.sync_info = mybir.SyncInfo(on_wait=[w], on_update=[])
                        out.append(nop)
                    inst.sync_info = mybir.SyncInfo(
                        on_wait=[waits[-1]],
                        on_update=list(si.on_update) if si and si.on_update else [])
                    changed = True
                out.append(inst)
            if changed:
                bb.instructions = out


def _build():
    import concourse.bass as bass
    import concourse.tile as tile
    from concourse import mybir
    from contextlib import ExitStack

    f32r = mybir.dt.float32r
    f32 = mybir.dt.float32
    f8 = mybir.dt.float8e4
    bf = mybir.dt.bfloat16
    u8 = mybir.dt.uint8
    u16 = mybir.dt.uint16
    AF = mybir.ActivationFunctionType
    DR = mybir.MatmulPerfMode.DoubleRow
    MUL = mybir.AluOpType.mult
    ADD = mybir.AluOpType.add

    nc = bass.Bass(trn_type="TRN2", target_bir_lowering=False, debug=False)

    x = nc.dram_tensor("x", [C, N], f32, kind="ExternalInput").ap()
    # packed fp8 weights: planes 0-3 wqk8a, 4-7 wqk8b, 8-11 wo8, 12-13 ones
    w8pk = nc.dram_tensor("w8pk", [128, 14, 512], u8, kind="ExternalInput").ap()
    wvpk = nc.dram_tensor("wvpk", [128, 4, 512], u16, kind="ExternalInput").ap()
    # packed per-channel consts: cols 0-3 16*gamma, 4-7 gamma/128,
    # 8-11 16*beta, 12-15 A_QK*hq/16, 16-19 bv, 20-23 bo, 24-31 gmask
    blpk = nc.dram_tensor("blpk", [128, 32], f32, kind="ExternalInput").ap()
    bmask = nc.dram_tensor("bmask", [8, 128], f32, kind="ExternalInput").ap()
    out = nc.dram_tensor("out", [C, NQ], f32, kind="ExternalOutput").ap()

    dma = nc.sync.dma_start
    qi_ch = 0  # host permutes x so the query quarter is chunk 0

    with tile.TileContext(nc) as tc, ExitStack() as top:
        consts = top.enter_context(tc.tile_pool(name="consts", bufs=1))
        xpool = top.enter_context(tc.tile_pool(name="xpool", bufs=1))
        x8pool = top.enter_context(tc.tile_pool(name="x8pool", bufs=1))
        wpool = top.enter_context(tc.tile_pool(name="wpool", bufs=1))
        w8pool = top.enter_context(tc.tile_pool(name="w8pool", bufs=1))
        qk8p = top.enter_context(tc.tile_pool(name="qk8p", bufs=1))
        xqsp = top.enter_context(tc.tile_pool(name="xqsp", bufs=1))
        v8p = top.enter_context(tc.tile_pool(name="v8p", bufs=1))
        ptp = top.enter_context(tc.tile_pool(name="ptp", bufs=1))
        spool = top.enter_context(tc.tile_pool(name="spool", bufs=1))
        rp = top.enter_context(tc.tile_pool(name="rp", bufs=2))
        osbp = top.enter_context(tc.tile_pool(name="osbp", bufs=2))
        outp = top.enter_context(tc.tile_pool(name="outp", bufs=4))
        # PSUM: 1 (scratch chain) + 4 (V/O pairs) + 3 (score rotation)
        ps_q = top.enter_context(tc.tile_pool(name="ps_q", bufs=1, space="PSUM"))
        ps_v = top.enter_context(tc.tile_pool(name="ps_v", bufs=2, space="PSUM"))
        ps_st = top.enter_context(tc.tile_pool(name="ps_st", bufs=3, space="PSUM"))

        def qtile(shape, name):
            return ps_q.tile(shape, f32, tag="q", name=name)

        def vtile(name):
            return ps_v.tile([128, 2, 512], f32, tag="pvt", name=name)

        def sttile(name):
            return ps_st.tile([128, 512], f32, tag="st", name=name)

        # ---- packed constant tiles (DMAs emitted after x below) ----
        ballc = consts.tile([128, 32], f32, tag="ballc", name="ballc")
        bm = consts.tile([8, 128], f32, tag="bm", name="bm")
        w8all = w8pool.tile([128, 14, 512], f8, tag="w8all", name="w8all")
        gam16 = [ballc[:, i:i + 1] for i in range(4)]
        gamdq = [ballc[:, 4 + i:5 + i] for i in range(4)]
        bet16 = [ballc[:, 8 + i:9 + i] for i in range(4)]
        hqc16 = [ballc[:, 12 + i:13 + i] for i in range(4)]
        bvc = [ballc[:, 16 + i:17 + i] for i in range(4)]
        boc = [ballc[:, 20 + i:21 + i] for i in range(4)]
        gm = ballc[:, 24:32]
        wqa = lambda p: w8all[:, 2 * p:2 * p + 2, :]          # noqa: E731
        wqb = lambda p: w8all[:, 4 + 2 * p:6 + 2 * p, :]      # noqa: E731
        wo_ = lambda p: w8all[:, 8 + 2 * p:10 + 2 * p, :]     # noqa: E731
        ones8 = w8all[:, 12:14, 0:128]
        epst = consts.tile([128, 1], f32, tag="epst", name="epst")
        nc.vector.memset(epst[:], EPS)
        shiftb = consts.tile([128, 1], f32, tag="shiftb", name="shiftb")
        nc.vector.memset(shiftb[:], -SHIFT)

        # ---- x resident (DMA) + fp8 copy (Pool) + stats (DVE) ----
        x_t = [[xpool.tile([128, 1024], f32, tag=f"x{ci}_{ch}", name=f"x{ci}_{ch}")
                for ch in range(4)] for ci in range(NCT)]
        x8t = [[x8pool.tile([128, 2, 1024], f8, tag=f"x8{p}_{ch}", name=f"x8{p}_{ch}")
                for ch in range(4)] for p in range(NCP)]
        stats3 = [spool.tile([128, 8, 6], f32, tag=f"st3s{i}", name=f"st3s{i}")
                  for i in range(NCT)]
        stats2 = [spool.tile([128, 2], f32, tag=f"st2{i}", name=f"st2{i}") for i in range(NCT)]
        ssum = qtile([8, 8], "ssum")
        dma(ballc[:], blpk)
        dma(bm[:], bmask)
        for ci in range(NCT):
            for ch in range(4):
                dma(x_t[ci][ch][:], x[ci * 128:(ci + 1) * 128,
                                      ch * 1024:(ch + 1) * 1024])
                nc.gpsimd.tensor_copy(x8t[ci // 2][ch][:, ci % 2, :],
                                      x_t[ci][ch][:])
                for h in range(2):
                    nc.vector.bn_stats(
                        out=stats3[ci][:, ch * 2 + h, :],
                        in_=x_t[ci][ch][:, h * 512:(h + 1) * 512])
            # per-ci stats head pipelined against remaining x DMAs
            mv = spool.tile([128, 2], f32, tag="mv", name="mv")
            nc.vector.bn_aggr(out=mv[:], in_=stats3[ci][:, :, :])
            m2 = spool.tile([128, 1], f32, tag="m2", name="m2")
            nc.vector.tensor_mul(m2[:], mv[:, 0:1], mv[:, 0:1])
            nc.vector.tensor_add(stats2[ci][:, 1:2], mv[:, 1:2], m2[:])
            nc.vector.tensor_copy(stats2[ci][:, 0:1], mv[:, 0:1])
            nc.tensor.matmul(ssum[0:8, 2 * ci:2 * ci + 2], gm,
                             stats2[ci][:], start=True, stop=True)
        wvall = wpool.tile([128, 4, 512], bf, tag="wvall", name="wvall")
        dma(wvall[:], wvpk.bitcast(bf))
        wv_t = [wvall[:, i, :] for i in range(NCT)]
        dma(w8all[:], w8pk.bitcast(f8))

        # ---- P1: group stats -> per-channel scale/shift ----
        sg = spool.tile([8, 8], f32, tag="sg", name="sg")
        nc.scalar.activation(sg[:], ssum[:], AF.Copy)
        m2g = spool.tile([8, 4], f32, tag="m2g", name="m2g")
        nc.vector.tensor_mul(m2g[:], sg[:, 0:8:2], sg[:, 0:8:2])
        varg = spool.tile([8, 4], f32, tag="varg", name="varg")
        nc.vector.tensor_sub(varg[:], sg[:, 1:8:2], m2g[:])
        sq = spool.tile([8, 4], f32, tag="sq", name="sq")
        nc.scalar.activation(sq[:], varg[:], AF.Sqrt, bias=epst[0:8, :],
                             scale=1.0)
        r0 = spool.tile([8, 4], f32, tag="r0", name="r0")
        nc.vector.reciprocal(r0[:], sq[:])
        mrstd = spool.tile([8, 8], f32, tag="mrstd", name="mrstd")
        nc.vector.tensor_copy(mrstd[:, 0:8:2], sg[:, 0:8:2])
        nc.vector.tensor_copy(mrstd[:, 1:8:2], r0[:])
        s16 = [spool.tile([128, 1], f32, tag=f"s16_{i}", name=f"s16_{i}") for i in range(NCT)]
        sdq = [spool.tile([128, 1], f32, tag=f"sdq{i}", name=f"sdq{i}") for i in range(NCT)]
        t2b = [spool.tile([128, 2], bf, tag=f"t2b{i}", name=f"t2b{i}") for i in range(NCT)]
        t16 = [spool.tile([128, 1], f32, tag=f"t16_{i}", name=f"t16_{i}") for i in range(NCT)]
        suq = [spool.tile([128, 1], f32, tag=f"suq{i}", name=f"suq{i}") for i in range(NCT)]
        for ci in range(NCT):
            pc_ps = qtile([128, 2], f"pc{ci}")
            nc.tensor.matmul(pc_ps[:], bm[:], mrstd[:, 2 * ci:2 * ci + 2],
                             start=True, stop=True)
            perch = spool.tile([128, 2], f32, tag=f"pch{ci}", name=f"pch{ci}")
            nc.scalar.activation(perch[:], pc_ps[:], AF.Copy)
            # s16 = A_XQS*s (== A_WV*s), sdq = s*A_QK/(A_WQK*A_XQS): host
            # prescales gamma so each is one ACT op off the rstd column
            nc.scalar.activation(s16[ci][:], perch[:, 1:2], AF.Identity,
                                 scale=gam16[ci])
            nc.scalar.activation(sdq[ci][:], perch[:, 1:2], AF.Identity,
                                 scale=gamdq[ci])
            tmp = spool.tile([128, 1], f32, tag="tmp1", name="tmp1")
            nc.vector.tensor_mul(tmp[:], perch[:, 0:1], s16[ci][:])
            nc.vector.tensor_sub(t16[ci][:], bet16[ci], tmp[:])
            nc.vector.tensor_mul(suq[ci][:], hqc16[ci], s16[ci][:])
            # t2b (bf16 GN shift for the off-critical bvt fold) on Pool
            nc.gpsimd.tensor_scalar_mul(t2b[ci][:, 0:1], t16[ci][:], 1.0 / A_XQS)
            nc.gpsimd.tensor_scalar_mul(t2b[ci][:, 1:2], t16[ci][:], 1.0 / A_XQS)

        # ---- query-side casts + qk projection (per i-half) ----
        xqs = [[xqsp.tile([128, 2, 512], f8, tag=f"xqs{p}_{ih}",
                          name=f"xqs{p}_{ih}") for ih in range(2)]
               for p in range(NCP)]
        qk8 = [[qk8p.tile([128, 2, 512], f8, tag=f"qk8{p}_{ih}",
                          name=f"qk8{p}_{ih}") for ih in range(2)]
               for p in range(NCP)]
        # xqs = A_XQS * (s*xq + t): ih0 cp0 on ACT, cp1 on Pool (parallel
        # head); critical path runs to the first exp
        for ci in range(2):
            nc.scalar.activation(xqs[0][0][:, ci, :],
                                 x_t[ci][qi_ch][:, 0:512],
                                 AF.Identity, bias=t16[ci][:],
                                 scale=s16[ci][:])
        for ci in range(2, NCT):
            nc.gpsimd.tensor_scalar(out=xqs[1][0][:, ci % 2, :],
                                    in0=x_t[ci][qi_ch][:, 0:512],
                                    scalar1=s16[ci][:], scalar2=t16[ci][:],
                                    op0=MUL, op1=ADD)
        # i1-half query cast and V-weight cast on the otherwise-idle Pool
        # engine; DVE carries only qk8-ih1 + the 16 V copies through phase B.
        wv8 = [w8pool.tile([128, 2, 512], f8, tag=f"wv8{p}", name=f"wv8{p}")
               for p in range(NCP)]
        for ci in range(NCT):
            nc.gpsimd.tensor_scalar_mul(wv8[ci // 2][:, ci % 2, :],
                                        wv_t[ci], s16[ci][:])
        for ci in range(NCT):
            nc.gpsimd.tensor_scalar(out=xqs[ci // 2][1][:, ci % 2, :],
                                    in0=x_t[ci][qi_ch][:, 512:1024],
                                    scalar1=s16[ci][:], scalar2=t16[ci][:],
                                    op0=MUL, op1=ADD)

        def qk_mms(ih):
            # pqk rides the 3-bank score-tile rotation (idle pre-scores) so
            # the four mt projections requantize concurrently
            for mt in range(NCT):
                m_sl = slice(mt * 128, (mt + 1) * 128)
                pqk = sttile(f"pqk{ih}_{mt}")
                k = 0
                for p in range(NCP):
                    for wt in (wqa, wqb):
                        nc.tensor.matmul(pqk[:], wt(p)[:, :, m_sl],
                                         xqs[p][ih][:],
                                         start=(k == 0), stop=(k == 3),
                                         perf_mode=DR)
                        k += 1
                if ih == 0 and mt < 2:
                    nc.scalar.activation(qk8[mt // 2][0][:, mt % 2, :],
                                         pqk[:], AF.Identity,
                                         bias=suq[mt][:], scale=sdq[mt][:])
                else:
                    nc.vector.tensor_scalar(
                        out=qk8[mt // 2][ih][:, mt % 2, :], in0=pqk[:],
                        scalar1=sdq[mt][:], scalar2=suq[mt][:],
                        op0=MUL, op1=ADD)

        # ---- V phase + scores/exp, interleaved emission ----
        v8 = [v8p.tile([128, 2, 512], f8, tag=f"v8_{jp}", name=f"v8_{jp}")
              for jp in range(NJP)]
        pt8 = [[ptp.tile([128, 2, 512], f8, tag=f"pt{ic}_{jp}",
                         name=f"pt{ic}_{jp}") for jp in range(NJP)]
               for ic in range(2)]

        pvts = {}

        def v_mms(jp):
            j0 = jp * 256
            ch, jj0 = j0 // 1024, j0 % 1024
            pvt = pvts[jp] = vtile(f"pvt{jp}")
            for jt in range(2):
                jsl = slice(jj0 + jt * 128, jj0 + jt * 128 + 128)
                for p in range(NCP):
                    nc.tensor.matmul(pvt[:, jt, :], x8t[p][ch][:, :, jsl],
                                     wv8[p][:], start=(p == 0),
                                     stop=(p == NCP - 1), perf_mode=DR)

        def v_copy(jp):
            nc.vector.tensor_copy(v8[jp][:], pvts.pop(jp)[:])

        def se_jp(ic, jp):
            j0 = jp * 256
            ch, jj0 = j0 // 1024, j0 % 1024
            for jt in range(2):
                st = sttile(f"s{ic}_{jp}_{jt}")
                jsl = slice(jj0 + jt * 128, jj0 + jt * 128 + 128)
                for p in range(NCP):
                    nc.tensor.matmul(st[:], x8t[p][ch][:, :, jsl],
                                     qk8[p][ic][:],
                                     start=(p == 0), stop=(p == NCP - 1),
                                     perf_mode=DR)
                nc.scalar.activation(pt8[ic][jp][:, jt, :], st[:],
                                     AF.Exp, bias=shiftb[:],
                                     scale=1.0 / (A_XJ * A_QK))

        # V lags the score stream by 4 j-pairs so the pvt double-buffer's
        # DVE copies never gate the PE->ACT exp cadence.
        qk_mms(0)
        for jp in range(NJP):
            se_jp(0, jp)
            k = jp - 4
            if k >= 0:
                v_mms(k)
                v_copy(k)
            if jp == 6:
                qk_mms(1)
        for k in range(NJP - 4, NJP):
            v_mms(k)
            v_copy(k)

        # ---- bvt / bo folds (PE here; elementwise on DVE) ----
        bvt_s = [spool.tile([128, 2], f32, tag=f"bvs{i}", name=f"bvs{i}") for i in range(NCT)]
        bvt8 = [spool.tile([128, 2, 1], f8, tag=f"bv8{p}", name=f"bv8{p}") for p in range(NCP)]
        bo_s = [spool.tile([128, 1], f32, tag=f"bos{i}", name=f"bos{i}") for i in range(NCT)]
        for mt in range(NCT):
            m_sl = slice(mt * 128, (mt + 1) * 128)
            pv = qtile([128, 2], f"pv{mt}")
            for ci in range(NCT):
                nc.tensor.matmul(pv[:], wv_t[ci][:, m_sl], t2b[ci][:],
                                 start=(ci == 0), stop=(ci == NCT - 1))
            nc.vector.tensor_scalar_add(bvt_s[mt][:], pv[:], bvc[mt])
            nc.vector.tensor_scalar_mul(bvt8[mt // 2][:, mt % 2, :],
                                        bvt_s[mt][:, 0:1], A_BVT)
        for ot in range(NCT):
            o_sl = slice(ot * 128, (ot + 1) * 128)
            pb = qtile([128, 1], f"pb{ot}")
            for p in range(NCP):
                nc.tensor.matmul(pb[:], wo_(p)[:, :, o_sl], bvt8[p][:],
                                 start=(p == 0), stop=(p == NCP - 1),
                                 perf_mode=DR)
            nc.vector.tensor_scalar(out=bo_s[ot][:], in0=pb[:, 0:1],
                                    scalar1=1.0 / (A_WO * A_BVT),
                                    scalar2=boc[ot], op0=MUL, op1=ADD)

        # ---- phase C: z(ic0), O(ic0) + scores/exp(ic1) ----
        oAB = [vtile("oA"), vtile("oB")]
        o_ps0 = [oAB[mt // 2][:, mt % 2, :] for mt in range(NCT)]
        z0 = qtile([128, 512], "z0")
        rinv0 = rp.tile([128, 512], f32, tag="rinv", name="rinv0")
        for jp in range(NJP):
            se_jp(1, jp)
            for mt in range(NCT):
                nc.tensor.matmul(o_ps0[mt], v8[jp][:, :, mt * 128:(mt + 1) * 128],
                                 pt8[0][jp][:], start=(jp == 0),
                                 stop=(jp == NJP - 1), perf_mode=DR)
            if jp == 2:
                # emitted mid-C so the PE burst rides C's slack instead of
                # delaying the first ic1 scores at the phase boundary
                for zj in range(NJP):
                    nc.tensor.matmul(z0[:], ones8, pt8[0][zj][:],
                                     start=(zj == 0), stop=(zj == NJP - 1),
                                     perf_mode=DR)
                with nc.allow_low_precision(reason="softmax reciprocal"):
                    nc.vector.reciprocal(rinv0[:], z0[:])

        # ---- phase D: z(ic1), O(ic1), epilogues ----
        zt1 = sttile("zt1")
        for jp in range(NJP):
            nc.tensor.matmul(zt1[:], ones8, pt8[1][jp][:],
                             start=(jp == 0), stop=(jp == NJP - 1),
                             perf_mode=DR)
        rinv1 = rp.tile([128, 512], f32, tag="rinv", name="rinv1")
        with nc.allow_low_precision(reason="softmax reciprocal"):
            nc.vector.reciprocal(rinv1[:], zt1[:])
        oCD = [vtile("oC"), vtile("oD")]
        o_ps1 = [oCD[mt // 2][:, mt % 2, :] for mt in range(NCT)]

        def o1_jp(jp):
            for mt in range(NCT):
                nc.tensor.matmul(o_ps1[mt], v8[jp][:, :, mt * 128:(mt + 1) * 128],
                                 pt8[1][jp][:], start=(jp == 0),
                                 stop=(jp == NJP - 1), perf_mode=DR)

        def epilogue(ic, o_ps, rinv, resid_eng):
            i0 = ic * 512
            osb = [osbp.tile([128, 2, 512], f8, tag=f"osb{p}", name=f"osb{ic}{p}")
                   for p in range(NCP)]
            for mt in range(NCT):
                nc.vector.tensor_mul(osb[mt // 2][:, mt % 2, :],
                                     o_ps[mt], rinv[:])
            for ot in range(NCT):
                o_sl = slice(ot * 128, (ot + 1) * 128)
                f_ps = sttile(f"f{ic}_{ot}")
                for p in range(NCP):
                    nc.tensor.matmul(f_ps[:], wo_(p)[:, :, o_sl], osb[p][:],
                                     start=(p == 0), stop=(p == NCP - 1),
                                     perf_mode=DR)
                ot_sb = outp.tile([128, 512], f32, tag="outsb", name="outsb")
                nc.scalar.activation(ot_sb[:], f_ps[:], AF.Identity,
                                     bias=bo_s[ot][:], scale=1.0 / (A_WO * A_O))
                resid_eng.tensor_add(ot_sb[:], ot_sb[:],
                                     x_t[ot][qi_ch][:, i0:i0 + 512])
                dma(out[ot * 128:(ot + 1) * 128, i0:i0 + 512], ot_sb[:])

        for jp in range(11):
            o1_jp(jp)
        epilogue(0, o_ps0, rinv0, nc.gpsimd)
        for jp in range(11, NJP):
            o1_jp(jp)
        epilogue(1, o_ps1, rinv1, nc.vector)

    _legalize_waits(nc, mybir)
    return nc


def kernel(**inputs):
    import concourse.bass  # noqa: F401
    from concourse.bass_utils import run_bass_kernel_spmd

    E4 = ml_dtypes.float8_e4m3
    BF = ml_dtypes.bfloat16

    x = np.asarray(inputs["x"], dtype=np.float32)
    gamma = np.asarray(inputs["gamma"], np.float32)
    beta = np.asarray(inputs["beta"], np.float32)
    wq = np.asarray(inputs["wq"], np.float32)
    bq = np.asarray(inputs["bq"], np.float32)
    wk = np.asarray(inputs["wk"], np.float32)
    wv = np.asarray(inputs["wv"], np.float32)
    wo = np.asarray(inputs["wo"], np.float32)
    bv = np.asarray(inputs["bv"], np.float32)
    bo = np.asarray(inputs["bo"], np.float32)

    Bb, Cc, H, W = x.shape
    scale = Cc ** (-0.5)
    xf = x.reshape(Bb, Cc, H * W)

    def pack_dr(w, alpha):
        """[C, 512] f32 -> [128, 2cp, 2t, 512] e4m3 planes (c = cp*256+t*128+p)."""
        q = np.clip(alpha * w, -240, 240).astype(E4)
        return q.reshape(2, 2, 128, 512).transpose(2, 0, 1, 3)  # [p, cp, t, m]

    wqk_h = scale * (wq.T @ wk)
    a8f = np.clip(A_WQK * wqk_h, -240, 240).astype(E4).astype(np.float32)
    w8pk = np.zeros((128, 14, 512), E4)
    w8pk[:, 0:4, :] = pack_dr(wqk_h, A_WQK).reshape(128, 4, 512)
    w8pk[:, 4:8, :] = (np.clip(A_WQK * wqk_h - a8f, -240, 240).astype(E4)
                       .reshape(2, 2, 128, 512).transpose(2, 0, 1, 3)
                       .reshape(128, 4, 512))
    w8pk[:, 8:12, :] = pack_dr(wo.T, A_WO).reshape(128, 4, 512)
    w8pk[:, 12:14, 0:128] = np.ones((128, 2, 128), np.float32).astype(E4)
    w8pk = np.ascontiguousarray(w8pk).view(np.uint8)

    wvpk = np.ascontiguousarray(
        wv.T.astype(BF).reshape(4, 128, 512).transpose(1, 0, 2)).view(np.uint16)

    blpk = np.zeros((128, 32), np.float32)
    blpk[:, 0:4] = (A_XQS * gamma).reshape(4, 128).T
    blpk[:, 4:8] = (gamma * (A_QK / (A_WQK * A_XQS))).reshape(4, 128).T
    blpk[:, 8:12] = (A_XQS * beta).reshape(4, 128).T
    blpk[:, 12:16] = ((A_QK / A_XQS) * scale * (wk.T @ bq)).reshape(4, 128).T
    blpk[:, 16:20] = bv.reshape(4, 128).T
    blpk[:, 20:24] = bo.reshape(4, 128).T
    for p in range(128):
        blpk[p, 24 + p // 16] = 1.0 / 16.0
    bmask = np.zeros((8, 128), np.float32)
    for p in range(128):
        bmask[p // 16, p] = 1.0

    common = {"w8pk": w8pk, "wvpk": wvpk, "blpk": blpk, "bmask": bmask}

    if "nc" not in _cache:
        _cache["nc"] = _build()
    nc = _cache["nc"]

    in_maps = []
    for core in range(8):
        b, qi = core // 4, core % 4
        xb = xf[b]
        q0 = qi * NQ
        xperm = np.ascontiguousarray(np.concatenate(
            [xb[:, q0:q0 + NQ], xb[:, :q0], xb[:, q0 + NQ:]], axis=1))
        in_maps.append({"x": xperm, **common})

    res = run_bass_kernel_spmd(nc, in_maps, core_ids=list(range(8)))
    outf = np.empty((Bb, Cc, H * W), np.float32)
    for core in range(8):
        b, qi = core // 4, core % 4
        outf[b][:, qi * NQ:(qi + 1) * NQ] = res.results[core]["out"]
    return outf.reshape(Bb, Cc, H, W)


# revision 45
# speedup vs baseline: 2.3665x; 1.0252x over previous
"""AttnBlock (GroupNorm -> single-head attention over H*W -> proj -> residual)
for Trainium2, 8 NeuronCores via SPMD.

Sharding: core = b*4 + qi (b = batch 0/1, qi = query-quarter 0..3).  The host
permutes each core's x columns so its query quarter is always column-chunk 0
(attention + GN stats are permutation-invariant over the key axis), so one
compiled kernel serves all 8 cores.

v3: every large matmul runs as fp8e4 (e4m3) with MatmulPerfMode.DoubleRow
(K=256 per instruction, 0.5 cycles/row).  Each fp8 tensor carries a
power-of-2 pre-scale placing it in e4m3's normal range (wq^T wk entries are
~2^-9 -- subnormal unscaled); inverse scales fold into activation/
tensor_scalar operands.  wqk = wq^T wk ships as a two-term fp8 sum
(value + quantization residual): it is the dominant quantization error
source.  Scores stay in [J, I] layout; the softmax denominator comes from an
all-ones fp8 DoubleRow matmul broadcasting z to all 128 partitions.  The
j-side GN affine cancels in softmax; the i-side affine folds into the qk
bias.  O accumulates over the full 4096-key sequence in PSUM.

Scheduling: one packed DMA per weight class (tiny SWDGE DMAs would occupy
the Pool engine for ~30us); score tiles rotate through a 3-buffer PSUM pool
(per-tile dep tracking pipelines PE scores against ACT exp); qk8/xqs split
per i-half so phase-B readers don't wait on phase-C writers; elementwise
work spread over ACT (exp, i0-half casts), DVE (stats, V copies, i1-half
casts, normalize), Pool (x->fp8 copies, residual adds).
"""
import sys

sys.path.insert(0, '/opt/trn_rl_repo')

import numpy as np
import ml_dtypes

C = 512
NG = 32
EPS = 1e-6
B = 2
N = 4096          # H*W
NQ = 1024         # query quarter
NCT = 4           # C // 128
NCP = 2           # C // 256 (DoubleRow c-pairs)
NJP = 16          # j-pairs of 256

SHIFT = 2.5
A_WQK = 1024.0
A_XJ = 1.0        # A_XJ * A_WV == A_V so the V cast is a pure copy
A_XQS = 16.0
A_QK = 128.0
A_WV = 16.0
A_V = 16.0
A_O = 16.0        # == A_V keeps the 1/z fold scale-free
A_T = 256.0
A_BVT = 256.0
A_WO = 64.0

_cache = {}


def _legalize_waits(nc, mybir):
    """Codegen allows exactly ONE sync wait per instruction. Hoist excess
    waits onto preceding same-engine NoOps (semantics preserving)."""
    gen = 0
    for f in nc.m.functions:
        for bb in f.blocks:
            insts = list(bb.instructions)
            out = []
            changed = False
            for inst in insts:
                si = inst.sync_info
                waits = list(si.on_wait) if si and si.on_wait else []
                if len(waits) > 1:
                    for w in waits[:-1]:
                        gen += 1
                        nop = mybir.InstNoOp(
                            name=f"waitnop_{gen}", ins=[], outs=[],
                            engine=inst.engine)
                        nop.sync_info = mybir.SyncInfo(on_wait=[w], on_update=[])
                        out.append(nop)
                    inst.sync_info = mybir.SyncInfo(
                        on_wait=[waits[-1]],
                        on_update=list(si.on_update) if si and si.on_update else [])
                    changed = True
                out.append(inst)
            if changed:
                bb.instructions = out


def _build():
    import concourse.bass as bass
    import concourse.tile as tile
    from concourse import mybir
    from contextlib import ExitStack

    f32r = mybir.dt.float32r
    f32 = mybir.dt.float32
    f8 = mybir.dt.float8e4
    bf = mybir.dt.bfloat16
    u8 = mybir.dt.uint8
    u16 = mybir.dt.uint16
    AF = mybir.ActivationFunctionType
    DR = mybir.MatmulPerfMode.DoubleRow
    MUL = mybir.AluOpType.mult
    ADD = mybir.AluOpType.add

    nc = bass.Bass(trn_type="TRN2", target_bir_lowering=False, debug=False)

    x = nc.dram_tensor("x", [C, N], f32, kind="ExternalInput").ap()
    # packed fp8 weights: planes 0-3 wqk8a, 4-7 wqk8b, 8-11 wo8, 12-13 ones
    w8pk = nc.dram_tensor("w8pk", [128, 14, 512], u8, kind="ExternalInput").ap()
    wvpk = nc.dram_tensor("wvpk", [128, 4, 512], u16, kind="ExternalInput").ap()
    # packed per-channel consts: cols 0-3 16*gamma, 4-7 gamma/128,
    # 8-11 16*beta, 12-15 A_QK*hq/16, 16-19 bv, 20-23 bo, 24-31 gmask
    blpk = nc.dram_tensor("blpk", [128, 32], f32, kind="ExternalInput").ap()
    bmask = nc.dram_tensor("bmask", [8, 128], f32, kind="ExternalInput").ap()
    out = nc.dram_tensor("out", [C, NQ], f32, kind="ExternalOutput").ap()

    dma = nc.sync.dma_start
    qi_ch = 0  # host permutes x so the query quarter is chunk 0

    with tile.TileContext(nc) as tc, ExitStack() as top:
        consts = top.enter_context(tc.tile_pool(name="consts", bufs=1))
        xpool = top.enter_context(tc.tile_pool(name="xpool", bufs=1))
        x8pool = top.enter_context(tc.tile_pool(name="x8pool", bufs=1))
        wpool = top.enter_context(tc.tile_pool(name="wpool", bufs=1))
        w8pool = top.enter_context(tc.tile_pool(name="w8pool", bufs=1))
        qk8p = top.enter_context(tc.tile_pool(name="qk8p", bufs=1))
        xqsp = top.enter_context(tc.tile_pool(name="xqsp", bufs=1))
        v8p = top.enter_context(tc.tile_pool(name="v8p", bufs=1))
        ptp = top.enter_context(tc.tile_pool(name="ptp", bufs=1))
        spool = top.enter_context(tc.tile_pool(name="spool", bufs=1))
        rp = top.enter_context(tc.tile_pool(name="rp", bufs=2))
        osbp = top.enter_context(tc.tile_pool(name="osbp", bufs=2))
        outp = top.enter_context(tc.tile_pool(name="outp", bufs=4))
        # PSUM: 4 banks (V/O pairs) + 4 banks (2 x 2-bank score slots);
        # every small scratch rides the score-slot rotation
        ps_v = top.enter_context(tc.tile_pool(name="ps_v", bufs=2, space="PSUM"))
        ps_st = top.enter_context(tc.tile_pool(name="ps_st", bufs=2, space="PSUM"))

        def qtile(shape, name):
            return ps_st.tile(shape, f32, tag="st", name=name)

        def vtile(name):
            return ps_v.tile([128, 2, 512], f32, tag="pvt", name=name)

        def sttile(name):
            return ps_st.tile([128, 1024], f32, tag="st", name=name)

        # ---- packed constant tiles (DMAs emitted after x below) ----
        ballc = consts.tile([128, 32], f32, tag="ballc", name="ballc")
        bm = consts.tile([8, 128], f32, tag="bm", name="bm")
        w8all = w8pool.tile([128, 14, 512], f8, tag="w8all", name="w8all")
        gam16 = [ballc[:, i:i + 1] for i in range(4)]
        gamdq = [ballc[:, 4 + i:5 + i] for i in range(4)]
        bet16 = [ballc[:, 8 + i:9 + i] for i in range(4)]
        hqc16 = [ballc[:, 12 + i:13 + i] for i in range(4)]
        bvc = [ballc[:, 16 + i:17 + i] for i in range(4)]
        boc = [ballc[:, 20 + i:21 + i] for i in range(4)]
        gm = ballc[:, 24:32]
        wqa = lambda p: w8all[:, 2 * p:2 * p + 2, :]          # noqa: E731
        wqb = lambda p: w8all[:, 4 + 2 * p:6 + 2 * p, :]      # noqa: E731
        wo_ = lambda p: w8all[:, 8 + 2 * p:10 + 2 * p, :]     # noqa: E731
        ones8 = w8all[:, 12:14, 0:128]
        epst = consts.tile([128, 1], f32, tag="epst", name="epst")
        nc.vector.memset(epst[:], EPS)
        shiftb = consts.tile([128, 1], f32, tag="shiftb", name="shiftb")
        nc.vector.memset(shiftb[:], -SHIFT)

        # ---- x resident (DMA) + fp8 copy (Pool) + stats (DVE) ----
        x_t = [[xpool.tile([128, 1024], f32, tag=f"x{ci}_{ch}", name=f"x{ci}_{ch}")
                for ch in range(4)] for ci in range(NCT)]
        x8t = [[x8pool.tile([128, 2, 1024], f8, tag=f"x8{p}_{ch}", name=f"x8{p}_{ch}")
                for ch in range(4)] for p in range(NCP)]
        stats3 = [spool.tile([128, 8, 6], f32, tag=f"st3s{i}", name=f"st3s{i}")
                  for i in range(NCT)]
        stats2 = [spool.tile([128, 2], f32, tag=f"st2{i}", name=f"st2{i}") for i in range(NCT)]
        ssum = qtile([8, 8], "ssum")
        dma(ballc[:], blpk)
        dma(bm[:], bmask)
        for ci in range(NCT):
            for ch in range(4):
                dma(x_t[ci][ch][:], x[ci * 128:(ci + 1) * 128,
                                      ch * 1024:(ch + 1) * 1024])
                nc.gpsimd.tensor_copy(x8t[ci // 2][ch][:, ci % 2, :],
                                      x_t[ci][ch][:])
                for h in range(2):
                    nc.vector.bn_stats(
                        out=stats3[ci][:, ch * 2 + h, :],
                        in_=x_t[ci][ch][:, h * 512:(h + 1) * 512])
            # per-ci stats head pipelined against remaining x DMAs
            mv = spool.tile([128, 2], f32, tag="mv", name="mv")
            nc.vector.bn_aggr(out=mv[:], in_=stats3[ci][:, :, :])
            m2 = spool.tile([128, 1], f32, tag="m2", name="m2")
            nc.vector.tensor_mul(m2[:], mv[:, 0:1], mv[:, 0:1])
            nc.vector.tensor_add(stats2[ci][:, 1:2], mv[:, 1:2], m2[:])
            nc.vector.tensor_copy(stats2[ci][:, 0:1], mv[:, 0:1])
            nc.tensor.matmul(ssum[0:8, 2 * ci:2 * ci + 2], gm,
                             stats2[ci][:], start=True, stop=True)
        wvall = wpool.tile([128, 4, 512], bf, tag="wvall", name="wvall")
        dma(wvall[:], wvpk.bitcast(bf))
        wv_t = [wvall[:, i, :] for i in range(NCT)]
        dma(w8all[:], w8pk.bitcast(f8))

        # ---- P1: group stats -> per-channel scale/shift ----
        sg = spool.tile([8, 8], f32, tag="sg", name="sg")
        nc.scalar.activation(sg[:], ssum[:], AF.Copy)
        m2g = spool.tile([8, 4], f32, tag="m2g", name="m2g")
        nc.vector.tensor_mul(m2g[:], sg[:, 0:8:2], sg[:, 0:8:2])
        varg = spool.tile([8, 4], f32, tag="varg", name="varg")
        nc.vector.tensor_sub(varg[:], sg[:, 1:8:2], m2g[:])
        sq = spool.tile([8, 4], f32, tag="sq", name="sq")
        nc.scalar.activation(sq[:], varg[:], AF.Sqrt, bias=epst[0:8, :],
                             scale=1.0)
        r0 = spool.tile([8, 4], f32, tag="r0", name="r0")
        nc.vector.reciprocal(r0[:], sq[:])
        mrstd = spool.tile([8, 8], f32, tag="mrstd", name="mrstd")
        nc.vector.tensor_copy(mrstd[:, 0:8:2], sg[:, 0:8:2])
        nc.vector.tensor_copy(mrstd[:, 1:8:2], r0[:])
        s16 = [spool.tile([128, 1], f32, tag=f"s16_{i}", name=f"s16_{i}") for i in range(NCT)]
        sdq = [spool.tile([128, 1], f32, tag=f"sdq{i}", name=f"sdq{i}") for i in range(NCT)]
        t2b = [spool.tile([128, 2], bf, tag=f"t2b{i}", name=f"t2b{i}") for i in range(NCT)]
        t16 = [spool.tile([128, 1], f32, tag=f"t16_{i}", name=f"t16_{i}") for i in range(NCT)]
        suq = [spool.tile([128, 1], f32, tag=f"suq{i}", name=f"suq{i}") for i in range(NCT)]
        for ci in range(NCT):
            pc_ps = qtile([128, 2], f"pc{ci}")
            nc.tensor.matmul(pc_ps[:], bm[:], mrstd[:, 2 * ci:2 * ci + 2],
                             start=True, stop=True)
            perch = spool.tile([128, 2], f32, tag=f"pch{ci}", name=f"pch{ci}")
            nc.scalar.activation(perch[:], pc_ps[:], AF.Copy)
            # s16 = A_XQS*s (== A_WV*s), sdq = s*A_QK/(A_WQK*A_XQS): host
            # prescales gamma so each is one ACT op off the rstd column
            nc.scalar.activation(s16[ci][:], perch[:, 1:2], AF.Identity,
                                 scale=gam16[ci])
            nc.scalar.activation(sdq[ci][:], perch[:, 1:2], AF.Identity,
                                 scale=gamdq[ci])
            tmp = spool.tile([128, 1], f32, tag="tmp1", name="tmp1")
            nc.vector.tensor_mul(tmp[:], perch[:, 0:1], s16[ci][:])
            nc.vector.tensor_sub(t16[ci][:], bet16[ci], tmp[:])
            nc.vector.tensor_mul(suq[ci][:], hqc16[ci], s16[ci][:])
            # t2b (bf16 GN shift for the off-critical bvt fold) on Pool
            nc.gpsimd.tensor_scalar_mul(t2b[ci][:, 0:1], t16[ci][:], 1.0 / A_XQS)
            nc.gpsimd.tensor_scalar_mul(t2b[ci][:, 1:2], t16[ci][:], 1.0 / A_XQS)

        # ---- query-side casts + qk projection (per i-half) ----
        xqs = [[xqsp.tile([128, 2, 512], f8, tag=f"xqs{p}_{ih}",
                          name=f"xqs{p}_{ih}") for ih in range(2)]
               for p in range(NCP)]
        qk8 = [[qk8p.tile([128, 2, 512], f8, tag=f"qk8{p}_{ih}",
                          name=f"qk8{p}_{ih}") for ih in range(2)]
               for p in range(NCP)]
        # xqs = A_XQS * (s*xq + t): ih0 cp0 on ACT, cp1 on Pool (parallel
        # head); critical path runs to the first exp
        for ci in range(2):
            nc.scalar.activation(xqs[0][0][:, ci, :],
                                 x_t[ci][qi_ch][:, 0:512],
                                 AF.Identity, bias=t16[ci][:],
                                 scale=s16[ci][:])
        for ci in range(2, NCT):
            nc.gpsimd.tensor_scalar(out=xqs[1][0][:, ci % 2, :],
                                    in0=x_t[ci][qi_ch][:, 0:512],
                                    scalar1=s16[ci][:], scalar2=t16[ci][:],
                                    op0=MUL, op1=ADD)
        # i1-half query cast and V-weight cast on the otherwise-idle Pool
        # engine; DVE carries only qk8-ih1 + the 16 V copies through phase B.
        wv8 = [w8pool.tile([128, 2, 512], f8, tag=f"wv8{p}", name=f"wv8{p}")
               for p in range(NCP)]
        for ci in range(NCT):
            nc.gpsimd.tensor_scalar_mul(wv8[ci // 2][:, ci % 2, :],
                                        wv_t[ci], s16[ci][:])
        for ci in range(NCT):
            nc.gpsimd.tensor_scalar(out=xqs[ci // 2][1][:, ci % 2, :],
                                    in0=x_t[ci][qi_ch][:, 512:1024],
                                    scalar1=s16[ci][:], scalar2=t16[ci][:],
                                    op0=MUL, op1=ADD)

        def qk_mms(ih):
            # pqk rides the 3-bank score-tile rotation (idle pre-scores) so
            # the four mt projections requantize concurrently
            for mt in range(NCT):
                m_sl = slice(mt * 128, (mt + 1) * 128)
                pqk = qtile([128, 512], f"pqk{ih}_{mt}")
                k = 0
                for p in range(NCP):
                    for wt in (wqa, wqb):
                        nc.tensor.matmul(pqk[:], wt(p)[:, :, m_sl],
                                         xqs[p][ih][:],
                                         start=(k == 0), stop=(k == 3),
                                         perf_mode=DR)
                        k += 1
                if ih == 1 or mt < 2:
                    nc.scalar.activation(qk8[mt // 2][ih][:, mt % 2, :],
                                         pqk[:], AF.Identity,
                                         bias=suq[mt][:], scale=sdq[mt][:])
                else:
                    nc.vector.tensor_scalar(
                        out=qk8[mt // 2][ih][:, mt % 2, :], in0=pqk[:],
                        scalar1=sdq[mt][:], scalar2=suq[mt][:],
                        op0=MUL, op1=ADD)

        # ---- V phase + scores/exp, interleaved emission ----
        v8 = [v8p.tile([128, 2, 512], f8, tag=f"v8_{jp}", name=f"v8_{jp}")
              for jp in range(NJP)]
        pt8 = [[ptp.tile([128, 2, 512], f8, tag=f"pt{ic}_{jp}",
                         name=f"pt{ic}_{jp}") for jp in range(NJP)]
               for ic in range(2)]

        pvts = {}

        def v_mms(jp):
            j0 = jp * 256
            ch, jj0 = j0 // 1024, j0 % 1024
            pvt = pvts[jp] = vtile(f"pvt{jp}")
            for jt in range(2):
                jsl = slice(jj0 + jt * 128, jj0 + jt * 128 + 128)
                for p in range(NCP):
                    nc.tensor.matmul(pvt[:, jt, :], x8t[p][ch][:, :, jsl],
                                     wv8[p][:], start=(p == 0),
                                     stop=(p == NCP - 1), perf_mode=DR)

        def v_copy(jp):
            nc.vector.tensor_copy(v8[jp][:], pvts.pop(jp)[:])

        def se_jp(ic, jp):
            j0 = jp * 256
            ch, jj0 = j0 // 1024, j0 % 1024
            st = sttile(f"s{ic}_{jp}")
            for jt in range(2):
                jsl = slice(jj0 + jt * 128, jj0 + jt * 128 + 128)
                for p in range(NCP):
                    nc.tensor.matmul(st[:, jt * 512:(jt + 1) * 512],
                                     x8t[p][ch][:, :, jsl],
                                     qk8[p][ic][:],
                                     start=(p == 0), stop=(p == NCP - 1),
                                     perf_mode=DR)
            nc.scalar.activation(pt8[ic][jp][:], st[:], AF.Exp,
                                 bias=shiftb[:], scale=1.0 / (A_XJ * A_QK))

        # ---- bvt / bo folds (PE here; elementwise on DVE) ----
        bvt_s = [spool.tile([128, 2], f32, tag=f"bvs{i}", name=f"bvs{i}") for i in range(NCT)]
        bvt8 = [spool.tile([128, 2, 1], f8, tag=f"bv8{p}", name=f"bv8{p}") for p in range(NCP)]
        bo_s = [spool.tile([128, 1], f32, tag=f"bos{i}", name=f"bos{i}") for i in range(NCT)]
        for mt in range(NCT):
            m_sl = slice(mt * 128, (mt + 1) * 128)
            pv = qtile([128, 2], f"pv{mt}")
            for ci in range(NCT):
                nc.tensor.matmul(pv[:], wv_t[ci][:, m_sl], t2b[ci][:],
                                 start=(ci == 0), stop=(ci == NCT - 1))
            nc.vector.tensor_scalar_add(bvt_s[mt][:], pv[:], bvc[mt])
            nc.vector.tensor_scalar_mul(bvt8[mt // 2][:, mt % 2, :],
                                        bvt_s[mt][:, 0:1], A_BVT)
        for ot in range(NCT):
            o_sl = slice(ot * 128, (ot + 1) * 128)
            pb = qtile([128, 1], f"pb{ot}")
            for p in range(NCP):
                nc.tensor.matmul(pb[:], wo_(p)[:, :, o_sl], bvt8[p][:],
                                 start=(p == 0), stop=(p == NCP - 1),
                                 perf_mode=DR)
            nc.vector.tensor_scalar(out=bo_s[ot][:], in0=pb[:, 0:1],
                                    scalar1=1.0 / (A_WO * A_BVT),
                                    scalar2=boc[ot], op0=MUL, op1=ADD)

        # V lags the score stream by 4 j-pairs so the pvt double-buffer's
        # DVE copies never gate the PE->ACT exp cadence.
        qk_mms(0)
        for jp in range(NJP):
            se_jp(0, jp)
            k = jp - 4
            if k >= 0:
                v_mms(k)
                v_copy(k)
            if jp == 6:
                qk_mms(1)
        for k in range(NJP - 4, NJP):
            v_mms(k)
            v_copy(k)

        # ---- phase C: z(ic0), O(ic0) + scores/exp(ic1) ----
        oAB = [vtile("oA"), vtile("oB")]
        o_ps0 = [oAB[mt // 2][:, mt % 2, :] for mt in range(NCT)]
        z0 = qtile([128, 512], "z0")
        rinv0 = rp.tile([128, 512], f32, tag="rinv", name="rinv0")
        for jp in range(NJP):
            se_jp(1, jp)
            for mt in range(NCT):
                nc.tensor.matmul(o_ps0[mt], v8[jp][:, :, mt * 128:(mt + 1) * 128],
                                 pt8[0][jp][:], start=(jp == 0),
                                 stop=(jp == NJP - 1), perf_mode=DR)
            if jp == 2:
                # emitted mid-C so the PE burst rides C's slack instead of
                # delaying the first ic1 scores at the phase boundary
                for zj in range(NJP):
                    nc.tensor.matmul(z0[:], ones8, pt8[0][zj][:],
                                     start=(zj == 0), stop=(zj == NJP - 1),
                                     perf_mode=DR)
                with nc.allow_low_precision(reason="softmax reciprocal"):
                    nc.vector.reciprocal(rinv0[:], z0[:])

        # ---- phase D: z(ic1), O(ic1), epilogues ----
        zt1 = qtile([128, 512], "zt1")
        for jp in range(NJP):
            nc.tensor.matmul(zt1[:], ones8, pt8[1][jp][:],
                             start=(jp == 0), stop=(jp == NJP - 1),
                             perf_mode=DR)
        rinv1 = rp.tile([128, 512], f32, tag="rinv", name="rinv1")
        with nc.allow_low_precision(reason="softmax reciprocal"):
            nc.vector.reciprocal(rinv1[:], zt1[:])
        oCD = [vtile("oC"), vtile("oD")]
        o_ps1 = [oCD[mt // 2][:, mt % 2, :] for mt in range(NCT)]

        def o1_jp(jp):
            for mt in range(NCT):
                nc.tensor.matmul(o_ps1[mt], v8[jp][:, :, mt * 128:(mt + 1) * 128],
                                 pt8[1][jp][:], start=(jp == 0),
                                 stop=(jp == NJP - 1), perf_mode=DR)

        def epilogue(ic, o_ps, rinv, resid_eng):
            i0 = ic * 512
            osb = [osbp.tile([128, 2, 512], f8, tag=f"osb{p}", name=f"osb{ic}{p}")
                   for p in range(NCP)]
            for mt in range(NCT):
                nc.vector.tensor_mul(osb[mt // 2][:, mt % 2, :],
                                     o_ps[mt], rinv[:])
            for ot in range(NCT):
                o_sl = slice(ot * 128, (ot + 1) * 128)
                f_ps = qtile([128, 512], f"f{ic}_{ot}")
                for p in range(NCP):
                    nc.tensor.matmul(f_ps[:], wo_(p)[:, :, o_sl], osb[p][:],
                                     start=(p == 0), stop=(p == NCP - 1),
                                     perf_mode=DR)
                ot_sb = outp.tile([128, 512], f32, tag="outsb", name="outsb")
                nc.scalar.activation(ot_sb[:], f_ps[:], AF.Identity,
                                     bias=bo_s[ot][:], scale=1.0 / (A_WO * A_O))
                resid_eng.tensor_add(ot_sb[:], ot_sb[:],
                                     x_t[ot][qi_ch][:, i0:i0 + 512])
                dma(out[ot * 128:(ot + 1) * 128, i0:i0 + 512], ot_sb[:])

        for jp in range(11):
            o1_jp(jp)
        epilogue(0, o_ps0, rinv0, nc.gpsimd)
        for jp in range(11, NJP):
            o1_jp(jp)
        epilogue(1, o_ps1, rinv1, nc.vector)

    _legalize_waits(nc, mybir)
    return nc


def kernel(**inputs):
    import concourse.bass  # noqa: F401
    from concourse.bass_utils import run_bass_kernel_spmd

    E4 = ml_dtypes.float8_e4m3
    BF = ml_dtypes.bfloat16

    x = np.asarray(inputs["x"], dtype=np.float32)
    gamma = np.asarray(inputs["gamma"], np.float32)
    beta = np.asarray(inputs["beta"], np.float32)
    wq = np.asarray(inputs["wq"], np.float32)
    bq = np.asarray(inputs["bq"], np.float32)
    wk = np.asarray(inputs["wk"], np.float32)
    wv = np.asarray(inputs["wv"], np.float32)
    wo = np.asarray(inputs["wo"], np.float32)
    bv = np.asarray(inputs["bv"], np.float32)
    bo = np.asarray(inputs["bo"], np.float32)

    Bb, Cc, H, W = x.shape
    scale = Cc ** (-0.5)
    xf = x.reshape(Bb, Cc, H * W)

    def pack_dr(w, alpha):
        """[C, 512] f32 -> [128, 2cp, 2t, 512] e4m3 planes (c = cp*256+t*128+p)."""
        q = np.clip(alpha * w, -240, 240).astype(E4)
        return q.reshape(2, 2, 128, 512).transpose(2, 0, 1, 3)  # [p, cp, t, m]

    wqk_h = scale * (wq.T @ wk)
    a8f = np.clip(A_WQK * wqk_h, -240, 240).astype(E4).astype(np.float32)
    w8pk = np.zeros((128, 14, 512), E4)
    w8pk[:, 0:4, :] = pack_dr(wqk_h, A_WQK).reshape(128, 4, 512)
    w8pk[:, 4:8, :] = (np.clip(A_WQK * wqk_h - a8f, -240, 240).astype(E4)
                       .reshape(2, 2, 128, 512).transpose(2, 0, 1, 3)
                       .reshape(128, 4, 512))
    w8pk[:, 8:12, :] = pack_dr(wo.T, A_WO).reshape(128, 4, 512)
    w8pk[:, 12:14, 0:128] = np.ones((128, 2, 128), np.float32).astype(E4)
    w8pk = np.ascontiguousarray(w8pk).view(np.uint8)

    wvpk = np.ascontiguousarray(
        wv.T.astype(BF).reshape(4, 128, 512).transpose(1, 0, 2)).view(np.uint16)

    blpk = np.zeros((128, 32), np.float32)
    blpk[:, 0:4] = (A_XQS * gamma).reshape(4, 128).T
    blpk[:, 4:8] = (gamma * (A_QK / (A_WQK * A_XQS))).reshape(4, 128).T
    blpk[:, 8:12] = (A_XQS * beta).reshape(4, 128).T
    blpk[:, 12:16] = ((A_QK / A_XQS) * scale * (wk.T @ bq)).reshape(4, 128).T
    blpk[:, 16:20] = bv.reshape(4, 128).T
    blpk[:, 20:24] = bo.reshape(4, 128).T
    for p in range(128):
        blpk[p, 24 + p // 16] = 1.0 / 16.0
    bmask = np.zeros((8, 128), np.float32)
    for p in range(128):
        bmask[p // 16, p] = 1.0

    common = {"w8pk": w8pk, "wvpk": wvpk, "blpk": blpk, "bmask": bmask}

    if "nc" not in _cache:
        _cache["nc"] = _build()
    nc = _cache["nc"]

    in_maps = []
    for core in range(8):
        b, qi = core // 4, core % 4
        xb = xf[b]
        q0 = qi * NQ
        xperm = np.ascontiguousarray(np.concatenate(
            [xb[:, q0:q0 + NQ], xb[:, :q0], xb[:, q0 + NQ:]], axis=1))
        in_maps.append({"x": xperm, **common})

    res = run_bass_kernel_spmd(nc, in_maps, core_ids=list(range(8)))
    outf = np.empty((Bb, Cc, H * W), np.float32)
    for core in range(8):
        b, qi = core // 4, core % 4
        outf[b][:, qi * NQ:(qi + 1) * NQ] = res.results[core]["out"]
    return outf.reshape(Bb, Cc, H, W)
